# revision 13
# baseline (speedup 1.0000x reference)
"""Trainium2 Bass kernel for a 2-layer GAT (PyG GATConv, concat=False, 3 heads).

Strategy (8 NeuronCores, SPMD), v2 "aggregate-then-project":
  * The GAT projection is linear, so the weighted message sum commutes with
    it:  sum_e p_e (z_src @ W_h) = (sum_e p_e z_src) @ W_h.  Each core
    therefore aggregates RAW 768-col features per head and applies the
    projection once per destination tile afterwards.  Tensor FLOPs are
    unchanged but the gathered/AllGathered table shrinks 3x (2310 -> 774
    cols): row = [z(768) | al_src hi(3) | al_src lo(3)] bf16.
  * Layer 1's table (x + its attention logits, fp32 on host, hi/lo bf16
    pair) is fully precomputed on the host and staged replicated on every
    core -> layer 1 runs with NO collective at all.  Layer 2's table is
    built on device and AllGathered in NSPLIT row-chunks that overlap the
    layer-1 edge phase.
  * Nodes range-sharded across 8 cores; edges partitioned by destination
    into 128-node dst blocks (CPB padded 128-edge chunks); self-loops
    appended on host.  Per block: indirect-DMA gather of src rows,
    e = leaky_relu(al_s[src] + al_d[dst]) (al_d expanded per edge via a
    mask-transpose matmul), p = exp(e) unnormalized, s = M^T p by mask
    matmul; per head the aggregation is accumulated TRANSPOSED
    (aggT[ch,dst] += G_z_slice^T @ (M*p_h)) so it feeds the projection
    matmul as lhsT with no transposes; out = sum_h (aggT_h^T @ W_h) *
    1/(3 s_h) + bias (normalization and head-mean commute with the sums).
  * PSUM (8 banks): aggT [P,768]f32 x2 bufs (4 banks; also reused for the
    layer-boundary o-transposes), proj split 512+256 (2 banks), one shared
    "attn" tile holding all per-chunk al_dst expansions + the softmax
    denominator + the layer-2 logit accumulator (2 banks).
  * Software pipeline: stage A (gather/attention) of block b+1 is emitted
    before stage B (aggregate/project) of block b; the s-matmuls of b+1
    are emitted inside B(b) after the first head's aggregation; head
    accumulation interleaves with projections so the PE never idles on
    vector-engine reductions.

Self-contained: only numpy/ml_dtypes/concourse (environment packages).
"""

import os
from contextlib import ExitStack
from dataclasses import dataclass

import ml_dtypes
import numpy as np

import concourse.bass as bass
import concourse.mybir as mybir
import concourse.tile as tile
from concourse import bacc
from concourse.bass import IndirectOffsetOnAxis

F32 = mybir.dt.float32
BF16 = mybir.dt.bfloat16
I32 = mybir.dt.int32
AF = mybir.ActivationFunctionType
OP = mybir.AluOpType

P = 128
NEG_SLOPE = 0.2


@dataclass(frozen=True)
class Cfg:
    N: int = 50000           # nodes
    D: int = 768             # feature dim (= per-head channels C)
    H: int = 3               # heads
    C: int = 768             # per-head channels
    n_cores: int = 8
    NSPLIT: int = 7          # layer-2 allgather row-chunks (in node tiles)

    @property
    def HC(self):
        return self.H * self.C

    @property
    def ROW(self):
        return self.D + 2 * self.H  # z | hi_s | lo_s

    @property
    def SHARD(self):
        return self.N // self.n_cores

    @property
    def NT(self):
        return (self.SHARD + P - 1) // P

    @property
    def KT(self):
        return self.D // P

    @property
    def tile_splits(self):
        sizes = [9, 9, 8, 8, 7, 5, 2, 1]
        assert sum(sizes) == self.NT
        out, i = [], 0
        for s in sizes:
            out.append(list(range(i, i + s)))
            i += s
        return out

    @property
    def splits(self):
        out = []
        for t in self.tile_splits:
            r0 = int(t[0]) * P
            r1 = min(int(t[-1] + 1) * P, self.SHARD)
            out.append((r0, r1 - r0))
        return out


CFG = Cfg()


# ---------------------------------------------------------------- host prep


def _hi_lo(v):
    hi = v.astype(ml_dtypes.bfloat16)
    lo = (v - hi.astype(np.float32)).astype(ml_dtypes.bfloat16)
    return hi, lo


def _head_proj(W, a, cfg):
    """[D, H] with col h = W_h @ a[h]."""
    out = np.zeros((cfg.D, cfg.H), np.float32)
    for h in range(cfg.H):
        out[:, h] = W[:, h * cfg.C:(h + 1) * cfg.C] @ a[h]
    return out


def _hbf_pos(gid, cfg):
    """Map global node id -> row in the split-major allgathered table."""
    k = gid // cfg.SHARD
    r = gid % cfg.SHARD
    pos = np.zeros_like(gid)
    base = 0
    for (s0, sz) in cfg.splits:
        m = (r >= s0) & (r < s0 + sz)
        pos = np.where(m, base + k * sz + (r - s0), pos)
        base += cfg.n_cores * sz
    return pos


def _prep(x, edge_index, W1, a_src1, a_dst1, b1, W2, a_src2, a_dst2, b2, cfg):
    N, SHARD, NBLOCK, H = cfg.N, cfg.SHARD, cfg.NT, cfg.H
    src = np.concatenate([np.asarray(edge_index[0]), np.arange(N)]).astype(np.int64)
    dst = np.concatenate([np.asarray(edge_index[1]), np.arange(N)]).astype(np.int64)
    order = np.argsort(dst, kind="stable")
    src_s, dst_s = src[order], dst[order]

    cores = []
    CPB = 1
    for k in range(cfg.n_cores):
        lo, hi = k * SHARD, (k + 1) * SHARD
        a = np.searchsorted(dst_s, lo)
        b = np.searchsorted(dst_s, hi)
        s_k = src_s[a:b].astype(np.int64)
        d_k = (dst_s[a:b] - lo).astype(np.int64)
        deg = np.bincount(d_k, minlength=SHARD)
        csum = np.concatenate([[0], np.cumsum(deg)])
        for bi in range(NBLOCK):
            n1 = min((bi + 1) * P, SHARD)
            ecnt = int(csum[n1] - csum[bi * P])
            CPB = max(CPB, (ecnt + P - 1) // P)
        cores.append((s_k, d_k, csum))
    NCHUNK = NBLOCK * CPB

    xf = np.asarray(x, np.float32)
    W1f = np.asarray(W1, np.float32)
    W2f = np.asarray(W2, np.float32)
    Wa_s1 = _head_proj(W1f, np.asarray(a_src1, np.float32), cfg)
    Wa_d1 = _head_proj(W1f, np.asarray(a_dst1, np.float32), cfg)
    Wa_s2 = _head_proj(W2f, np.asarray(a_src2, np.float32), cfg)
    Wa_d2 = _head_proj(W2f, np.asarray(a_dst2, np.float32), cfg)

    # layer-1 table: [x | hi_s | lo_s]  (replicated to every core)
    als1 = xf @ Wa_s1                      # [N, H] fp32
    ald1 = xf @ Wa_d1                      # [N, H] fp32
    hi_s, lo_s = _hi_lo(als1)
    T1 = np.concatenate(
        [xf.astype(ml_dtypes.bfloat16), hi_s, lo_s], axis=1)
    T1 = np.ascontiguousarray(T1)          # [N, ROW]

    W2A = np.ascontiguousarray(
        np.concatenate([Wa_s2, Wa_d2], axis=1).astype(ml_dtypes.bfloat16))
    W1K = np.ascontiguousarray(W1f.astype(ml_dtypes.bfloat16))
    W2K = np.ascontiguousarray(W2f.astype(ml_dtypes.bfloat16))
    B1 = np.ascontiguousarray(
        np.broadcast_to(np.asarray(b1, np.float32), (P, cfg.C)))
    B2 = np.ascontiguousarray(
        np.broadcast_to(np.asarray(b2, np.float32), (P, cfg.C)))

    hi_d, lo_d = _hi_lo(ald1)

    in_maps = []
    for k, (s_k, d_k, csum) in enumerate(cores):
        srcg1 = np.zeros((NCHUNK, P), np.int32)
        srcg2 = np.zeros((NCHUNK, P), np.int32)
        MMc = np.zeros((NBLOCK, P, 2 * CPB * P), ml_dtypes.bfloat16)
        s_pos2 = _hbf_pos(s_k, cfg).astype(np.int32)
        s_pos1 = s_k.astype(np.int32)
        for bi in range(NBLOCK):
            n1 = min((bi + 1) * P, SHARD)
            e0, e1_ = int(csum[bi * P]), int(csum[n1])
            ecnt = e1_ - e0
            ed = d_k[e0:e1_] - bi * P
            for srcg, s_pos in ((srcg1, s_pos1), (srcg2, s_pos2)):
                eb = np.zeros(CPB * P, np.int32)
                eb[:ecnt] = s_pos[e0:e1_]
                srcg[bi * CPB:(bi + 1) * CPB] = eb.reshape(CPB, P)
            Mb = np.zeros((CPB * P, P), ml_dtypes.bfloat16)
            Mb[np.arange(ecnt), ed] = 1.0
            Mb3 = Mb.reshape(CPB, P, P)
            MMc[bi, :, :CPB * P] = Mb3.transpose(1, 0, 2).reshape(P, CPB * P)
            MMc[bi, :, CPB * P:] = Mb3.transpose(2, 0, 1).reshape(P, CPB * P)
        # per-block al_dst hi/lo in [slot, block*2H + (hi|lo)] layout
        ALD = np.zeros((P, NBLOCK, 2 * H), ml_dtypes.bfloat16)
        r = np.arange(SHARD)
        ALD[r % P, r // P, :H] = hi_d[k * SHARD:(k + 1) * SHARD]
        ALD[r % P, r // P, H:] = lo_d[k * SHARD:(k + 1) * SHARD]
        in_maps.append({
            "T1": T1,
            "W1K": W1K, "W2K": W2K, "W2A": W2A, "B1": B1, "B2": B2,
            "SRC1": np.ascontiguousarray(srcg1.T),
            "SRC2": np.ascontiguousarray(srcg2.T),
            "ALD1": np.ascontiguousarray(ALD.reshape(P, NBLOCK * 2 * H)),
            "MMC": np.ascontiguousarray(
                MMc.transpose(1, 0, 2).reshape(P, NBLOCK * 2 * CPB * P)),
        })
    return in_maps, CPB


# ---------------------------------------------------------------- device code


def _build(cfg, CPB):
    NBLOCK = cfg.NT
    NCHUNK = NBLOCK * CPB
    D, C, H, HC, ROW = cfg.D, cfg.C, cfg.H, cfg.HC, cfg.ROW
    SHARD, NT, N, KT = cfg.SHARD, cfg.NT, cfg.N, cfg.KT
    MW = 2 * CPB * P
    AW = 2 * H * CPB + H     # attn tile cols: CPB aldt pairs | s

    nc = bacc.Bacc("TRN2", target_bir_lowering=False, debug=False,
                   num_devices=cfg.n_cores)

    T1 = nc.dram_tensor("T1", [N, ROW], BF16, kind="ExternalInput")
    W1K = nc.dram_tensor("W1K", [D, HC], BF16, kind="ExternalInput")
    W2K = nc.dram_tensor("W2K", [D, HC], BF16, kind="ExternalInput")
    W2A = nc.dram_tensor("W2A", [D, 2 * H], BF16, kind="ExternalInput")
    B1 = nc.dram_tensor("B1", [P, C], F32, kind="ExternalInput")
    B2 = nc.dram_tensor("B2", [P, C], F32, kind="ExternalInput")
    SRC1 = nc.dram_tensor("SRC1", [P, NCHUNK], I32, kind="ExternalInput")
    SRC2 = nc.dram_tensor("SRC2", [P, NCHUNK], I32, kind="ExternalInput")
    ALD1 = nc.dram_tensor("ALD1", [P, NBLOCK * 2 * H], BF16, kind="ExternalInput")
    MMC = nc.dram_tensor("MMC", [P, NBLOCK * MW], BF16, kind="ExternalInput")
    OUT = nc.dram_tensor("OUT", [SHARD, C], F32, kind="ExternalOutput")

    hbs2 = nc.dram_tensor("hbs2", [SHARD, ROW], BF16)
    hbf2 = nc.dram_tensor("hbf2", [N, ROW], BF16, addr_space="Shared")

    groups = [list(range(cfg.n_cores))]
    split_rows = cfg.splits
    split_bases = np.cumsum([0] + [cfg.n_cores * sz for (_, sz) in split_rows])
    last_tile_of_split = {ts[-1]: j for j, ts in enumerate(cfg.tile_splits)}

    with tile.TileContext(nc) as tc, ExitStack() as ctx:
        res = ctx.enter_context(tc.tile_pool(name="res", bufs=1))
        b1_sb = res.tile([P, C], F32, name="b1_sb")
        b2_sb = res.tile([P, C], F32, name="b2_sb")
        src1_sb = res.tile([P, NCHUNK], I32, name="src1_sb")
        nc.sync.dma_start(src1_sb[:], SRC1.ap())
        src2_sb = res.tile([P, NCHUNK], I32, name="src2_sb")
        nc.sync.dma_start(src2_sb[:], SRC2.ap())
        ald1 = res.tile([P, NBLOCK * 2 * H], BF16, name="ald1")
        nc.sync.dma_start(ald1[:], ALD1.ap())
        ald2 = res.tile([P, NBLOCK * 2 * H], BF16, name="ald2")
        nc.gpsimd.memset(ald2[:], 0.0)
        w2a_sb = res.tile([P, KT * 2 * H], BF16, name="w2a_sb")
        nc.sync.dma_start(
            w2a_sb[:].rearrange("p (t c) -> p t c", t=KT),
            W2A.ap().rearrange("(t p) c -> p t c", p=P))
        id_sb = res.tile([P, P], BF16, name="id_sb")
        from concourse.masks import make_identity
        make_identity(nc, id_sb[:])

        wp = ctx.enter_context(tc.tile_pool(name="wp", bufs=2))
        xp = ctx.enter_context(tc.tile_pool(name="xp", bufs=2))
        hp = ctx.enter_context(tc.tile_pool(name="hp", bufs=3))
        gp = ctx.enter_context(tc.tile_pool(name="gp", bufs=3))
        mp = ctx.enter_context(tc.tile_pool(name="mp", bufs=3))
        shp = ctx.enter_context(tc.tile_pool(name="shp", bufs=4))
        sp = ctx.enter_context(tc.tile_pool(name="sp", bufs=4))
        op_ = ctx.enter_context(tc.tile_pool(name="op", bufs=2))
        pp = ctx.enter_context(tc.tile_pool(name="pp", bufs=2, space="PSUM"))
        pj = ctx.enter_context(tc.tile_pool(name="pj", bufs=1, space="PSUM"))
        ea = ctx.enter_context(tc.tile_pool(name="ea", bufs=2, space="PSUM"))

        w_sbs = []

        def load_weights(L):
            nc.sync.dma_start((b1_sb if L == 0 else b2_sb)[:],
                              (B1 if L == 0 else B2).ap())
            w_sb = wp.tile([P, KT * HC], BF16, name=f"w_sb{L}", tag="W")
            W = W1K if L == 0 else W2K
            for kt in range(KT):
                nc.sync.dma_start(
                    w_sb[:, kt * HC:(kt + 1) * HC],
                    W.ap()[kt * P:(kt + 1) * P, :])
            w_sbs.append(w_sb)

        aldr_ = [ald1, ald2]
        src_ = [src1_sb, src2_sb]

        class St:
            pass

        def stage_a(L, b):
            """Gather + al_dst expansion + edge logits for block b."""
            st = St()
            st.b = b
            st.Mc = mp.tile([P, MW], BF16, name="Mc", tag="M", bufs=4)
            nc.sync.dma_start(st.Mc[:], MMC.ap()[:, b * MW:(b + 1) * MW])
            st.attn = ea.tile([P, AW], F32, name="attn", tag="attn", bufs=1)
            st.s_ps = ea.tile([P, H], F32, name="s_ps", tag="s", bufs=1)
            aldr = aldr_[L]
            tbl = T1 if L == 0 else hbf2
            st.Gs, st.pfs, st.pbs = [], [], []
            for cc in range(CPB):
                c = b * CPB + cc
                G = gp.tile([P, ROW], BF16, name="G", tag="G", bufs=3 * CPB + 4)
                nc.gpsimd.indirect_dma_start(
                    out=G[:], out_offset=None, in_=tbl.ap(),
                    in_offset=IndirectOffsetOnAxis(
                        ap=src_[L][:, c:c + 1], axis=0))
                ad = st.attn[:, cc * 2 * H:(cc + 1) * 2 * H]
                nc.tensor.matmul(
                    ad, lhsT=st.Mc[:, CPB * P + cc * P: CPB * P + (cc + 1) * P],
                    rhs=aldr[:, b * 2 * H:(b + 1) * 2 * H],
                    start=True, stop=True)
                e1 = sp.tile([P, H], F32, name="e1", tag="e1")
                nc.vector.tensor_tensor(
                    e1[:], G[:, D:D + H], G[:, D + H:D + 2 * H], op=OP.add)
                nc.vector.tensor_tensor(e1[:], e1[:], ad[:, 0:H], op=OP.add)
                nc.vector.tensor_tensor(e1[:], e1[:], ad[:, H:2 * H], op=OP.add)
                nc.vector.scalar_tensor_tensor(
                    e1[:], e1[:], NEG_SLOPE, e1[:], op0=OP.mult, op1=OP.max)
                pf = sp.tile([P, H], F32, name="pf", tag="pf", bufs=3 * CPB + 4)
                nc.scalar.activation(pf[:], e1[:], AF.Exp)
                pb = sp.tile([P, H], BF16, name="pb", tag="pb", bufs=2 * CPB + 2)
                nc.vector.tensor_copy(pb[:], pf[:])
                st.Gs.append(G)
                st.pfs.append(pf)
                st.pbs.append(pb)
            return st

        def stage_s(L, st):
            """Softmax denominator s = M^T p + 1/(3s) for block st.b."""
            s_ap = st.s_ps[:]
            for cc in range(CPB):
                nc.tensor.matmul(
                    s_ap, lhsT=st.Mc[:, cc * P:(cc + 1) * P], rhs=st.pbs[cc][:],
                    start=(cc == 0), stop=(cc == CPB - 1))

        def stage_r(L, st):
            """recip = 1/(3 s) (emitted at the end of the previous B)."""
            st.recip = sp.tile([P, H], F32, name="recip", tag="recip", bufs=2)
            nc.vector.tensor_scalar_mul(st.recip[:], st.s_ps[:], float(H))
            nc.vector.reciprocal(st.recip[:], st.recip[:])

        def stage_b(L, st, nxt, bnd=None):
            """Aggregate (transposed) + project + epilogue for block st.b."""
            b = st.b
            nw = min(P, SHARD - b * P)

            def emit_agg(h):
                # one accumulation group at a time per PSUM bank: kt outer,
                # cc inner (interleaved groups in one bank lose their first
                # chunk -- start=True clears has_written for the whole bank)
                ag = pp.tile([P, KT * P], F32, name="aggT", tag="aggT", bufs=2)
                Shs = []
                for cc in range(CPB):
                    Sh = shp.tile([P, P], BF16, name="Sh", tag="Sh", bufs=6)
                    nc.scalar.activation(
                        Sh[:], st.Mc[:, cc * P:(cc + 1) * P], AF.Copy,
                        scale=st.pfs[cc][:, h:h + 1])
                    Shs.append(Sh)
                for kt in range(KT):
                    for cc in range(CPB):
                        nc.tensor.matmul(
                            ag[:, kt * P:(kt + 1) * P],
                            lhsT=st.Gs[cc][:, kt * P:(kt + 1) * P], rhs=Shs[cc][:],
                            start=(cc == 0), stop=(cc == CPB - 1))
                return ag

            def emit_copy(ag):
                asb = op_.tile([P, KT * P], BF16, name="aggT_sb",
                               tag="aggT_sb", bufs=3)
                nc.scalar.copy(asb[:], ag[:])
                return asb

            def emit_proj(h, asb):
                pA = pj.tile([P, 512], F32, name="pA", tag="pA", bufs=1)
                pB = pj.tile([P, 256], F32, name="pB", tag="pB", bufs=1)
                for (pr, c0, cw) in ((pA, 0, 512), (pB, 512, 256)):
                    for kt in range(KT):
                        nc.tensor.matmul(
                            pr[:nw, :cw],
                            lhsT=asb[:, kt * P:kt * P + nw],
                            rhs=w_sbs[L][:, kt * HC + h * C + c0:
                                         kt * HC + h * C + c0 + cw],
                            start=(kt == 0), stop=(kt == KT - 1))
                return pA, pB

            bias_sb = b1_sb if L == 0 else b2_sb
            o = op_.tile([P, C], F32, name="o", tag="o", bufs=3)

            def emit_stt(h, pA, pB):
                for (pr, c0, cw) in ((pA, 0, 512), (pB, 512, 256)):
                    nc.vector.scalar_tensor_tensor(
                        o[:, c0:c0 + cw], pr[:, :cw], st.recip[:, h:h + 1],
                        bias_sb[:, c0:c0 + cw] if h == 0 else o[:, c0:c0 + cw],
                        op0=OP.mult, op1=OP.add)

            ag0 = emit_agg(0)
            ag1 = emit_agg(1)
            a0 = emit_copy(ag0)
            p0 = emit_proj(0, a0)
            emit_stt(0, *p0)
            ag2 = emit_agg(2)
            if L == 0 and nxt is not None:
                stage_s(L, nxt)
            a1 = emit_copy(ag1)
            p1 = emit_proj(1, a1)
            emit_stt(1, *p1)
            a2 = emit_copy(ag2)
            if L == 1 and nxt is not None:
                stage_s(L, nxt)
            if bnd is not None:
                emit_boundary(bnd)
            p2 = emit_proj(2, a2)
            emit_stt(2, *p2)
            if nxt is not None:
                stage_r(L, nxt)
            st.o = o

            if L == 1:
                nc.sync.dma_start(OUT.ap()[b * P:b * P + nw, :], o[:nw, :])

        def emit_boundary(st):
            """Layer-1 -> layer-2 transition for block st.b (deferred so the
            PE has work queued between proj2/stt2 and the transposes)."""
            b = st.b
            nw = min(P, SHARD - b * P)
            o = st.o
            if True:
                hb = hp.tile([P, ROW], BF16, name="hb", tag="hb", bufs=3)
                nc.scalar.activation(hb[:, 0:D], o[:], AF.Relu)
                lhsT2 = xp.tile([P, KT * P], BF16, name="lhsT2",
                                tag="lhsT2", bufs=2)
                for kt in range(KT):
                    tp = pp.tile([P, 2 * KT * P], BF16, name="tp", tag="aggT",
                                 bufs=2)
                    nc.tensor.transpose(
                        tp[:, 0:P], hb[:, kt * P:(kt + 1) * P], id_sb[:])
                    nc.scalar.copy(
                        lhsT2[:, kt * P:(kt + 1) * P], tp[:, 0:P])
                lg = ea.tile([P, AW], F32, name="lg", tag="attn", bufs=1)
                for kt in range(KT):
                    nc.tensor.matmul(
                        lg[:, 0:2 * H], lhsT=lhsT2[:, kt * P:(kt + 1) * P],
                        rhs=w2a_sb[:, kt * 2 * H:(kt + 1) * 2 * H],
                        start=(kt == 0), stop=(kt == KT - 1))
                nc.vector.tensor_copy(hb[:nw, D:D + H], lg[:nw, 0:H])
                nc.vector.tensor_tensor(
                    hb[:nw, D + H:D + 2 * H], lg[:nw, 0:H],
                    hb[:nw, D:D + H], op=OP.subtract)
                ao = b * 2 * H
                nc.vector.tensor_copy(ald2[:nw, ao:ao + H], lg[:nw, H:2 * H])
                nc.vector.tensor_tensor(
                    ald2[:nw, ao + H:ao + 2 * H], lg[:nw, H:2 * H],
                    ald2[:nw, ao:ao + H], op=OP.subtract)
                nc.sync.dma_start(hbs2.ap()[b * P:b * P + nw, :], hb[:nw, :])
                if b in last_tile_of_split:
                    j = last_tile_of_split[b]
                    s0, sz = split_rows[j]
                    nc.gpsimd.collective_compute(
                        "AllGather", OP.bypass, replica_groups=groups,
                        ins=[hbs2.ap()[s0:s0 + sz, :].opt()],
                        outs=[hbf2.ap()[int(split_bases[j]):
                                        int(split_bases[j + 1]), :].opt()])

        for L in range(2):
            prev, bnd = None, None
            for b in range(NBLOCK):
                cur = stage_a(L, b)
                if L == 0 and b == 0:
                    load_weights(0)
                if L == 0 and b == 2:
                    load_weights(1)
                if prev is None:
                    stage_s(L, cur)
                    stage_r(L, cur)
                else:
                    stage_b(L, prev, cur, bnd)
                    bnd = prev if L == 0 else None
                prev = cur
            stage_b(L, prev, None, bnd)
            if L == 0:
                emit_boundary(prev)

    nc.compile()
    return nc


# ---------------------------------------------------------------- entry point

_NC_CACHE = {}


def _get_nc(cfg, CPB):
    key = (cfg, CPB)
    if key not in _NC_CACHE:
        _NC_CACHE[key] = _build(cfg, CPB)
    return _NC_CACHE[key]


LAST_RUN = {}


def kernel(x, edge_index, W1, a_src1, a_dst1, b1, W2, a_src2, a_dst2, b2,
           cfg=CFG):
    from concourse.bass_utils import run_bass_kernel_spmd

    in_maps, CPB = _prep(x, edge_index, W1, a_src1, a_dst1, b1,
                         W2, a_src2, a_dst2, b2, cfg)
    nc = _get_nc(cfg, CPB)
    trace = os.environ.get("GAT_TRACE", "0") == "1"
    tmpdir = os.environ.get("GAT_TMPDIR") or None
    res = run_bass_kernel_spmd(nc, in_maps, list(range(cfg.n_cores)),
                               trace=trace, tmpdir=tmpdir)
    LAST_RUN["exec_time_ns"] = res.exec_time_ns
    LAST_RUN["profile_json"] = res.profile_json
    out = np.concatenate(
        [res.results[k]["OUT"] for k in range(cfg.n_cores)], 0)
    return np.ascontiguousarray(out.astype(np.float32))


# revision 15
# speedup vs baseline: 1.0078x; 1.0078x over previous
"""Trainium2 Bass kernel for a 2-layer GAT (PyG GATConv, concat=False, 3 heads).

Strategy (8 NeuronCores, SPMD), v2 "aggregate-then-project":
  * The GAT projection is linear, so the weighted message sum commutes with
    it:  sum_e p_e (z_src @ W_h) = (sum_e p_e z_src) @ W_h.  Each core
    therefore aggregates RAW 768-col features per head and applies the
    projection once per destination tile afterwards.  Tensor FLOPs are
    unchanged but the gathered/AllGathered table shrinks 3x (2310 -> 774
    cols): row = [z(768) | al_src hi(3) | al_src lo(3)] bf16.
  * Layer 1's table (x + its attention logits, fp32 on host, hi/lo bf16
    pair) is fully precomputed on the host and staged replicated on every
    core -> layer 1 runs with NO collective at all.  Layer 2's table is
    built on device and AllGathered in NSPLIT row-chunks that overlap the
    layer-1 edge phase.
  * Nodes range-sharded across 8 cores; edges partitioned by destination
    into 128-node dst blocks (CPB padded 128-edge chunks); self-loops
    appended on host.  Per block: indirect-DMA gather of src rows,
    e = leaky_relu(al_s[src] + al_d[dst]) (al_d expanded per edge via a
    mask-transpose matmul), p = exp(e) unnormalized, s = M^T p by mask
    matmul; per head the aggregation is accumulated TRANSPOSED
    (aggT[ch,dst] += G_z_slice^T @ (M*p_h)) so it feeds the projection
    matmul as lhsT with no transposes; out = sum_h (aggT_h^T @ W_h) *
    1/(3 s_h) + bias (normalization and head-mean commute with the sums).
  * PSUM (8 banks): aggT [P,768]f32 x2 bufs (4 banks; the tag is also
    reused for the layer-boundary transposes), projection split into
    512+256 col tiles (2 banks), one "attn" tile per block holding all
    per-chunk al_dst expansions (+ layer-2 logit accumulator via tag
    share, 1 bank), softmax denominator s in its own bank.  Only one
    matmul accumulation group is ever open per bank (start=True clears
    the has_written bits of the WHOLE bank).
  * Software pipeline: stage A (gather/attention) of block b+1 is
    emitted before stage B (aggregate/project) of block b; the s-matmuls
    of b+1 are emitted inside B(b) (after agg2 for layer 1, before
    proj2 for layer 2); the layer-1->2 boundary work of block b (relu,
    transposes, layer-2 logits, table-row DMA, AllGather chunks) is
    deferred into B(b+1) so the PE always has queued work while the
    epilogue drains; head accumulation interleaves with projections.
    Gathers/masks/logit chains prefetch 3-4 blocks ahead.

Self-contained: only numpy/ml_dtypes/concourse (environment packages).
"""

import os
from contextlib import ExitStack
from dataclasses import dataclass

import ml_dtypes
import numpy as np

import concourse.bass as bass
import concourse.mybir as mybir
import concourse.tile as tile
from concourse import bacc
from concourse.bass import IndirectOffsetOnAxis

F32 = mybir.dt.float32
BF16 = mybir.dt.bfloat16
I32 = mybir.dt.int32
AF = mybir.ActivationFunctionType
OP = mybir.AluOpType

P = 128
NEG_SLOPE = 0.2


@dataclass(frozen=True)
class Cfg:
    N: int = 50000           # nodes
    D: int = 768             # feature dim (= per-head channels C)
    H: int = 3               # heads
    C: int = 768             # per-head channels
    n_cores: int = 8
    NSPLIT: int = 7          # layer-2 allgather row-chunks (in node tiles)

    @property
    def HC(self):
        return self.H * self.C

    @property
    def ROW(self):
        return self.D + 2 * self.H  # z | hi_s | lo_s

    @property
    def SHARD(self):
        return self.N // self.n_cores

    @property
    def NT(self):
        return (self.SHARD + P - 1) // P

    @property
    def KT(self):
        return self.D // P

    @property
    def tile_splits(self):
        sizes = [9, 9, 8, 8, 7, 4, 3, 1]
        assert sum(sizes) == self.NT
        out, i = [], 0
        for s in sizes:
            out.append(list(range(i, i + s)))
            i += s
        return out

    @property
    def splits(self):
        out = []
        for t in self.tile_splits:
            r0 = int(t[0]) * P
            r1 = min(int(t[-1] + 1) * P, self.SHARD)
            out.append((r0, r1 - r0))
        return out


CFG = Cfg()


# ---------------------------------------------------------------- host prep


def _hi_lo(v):
    hi = v.astype(ml_dtypes.bfloat16)
    lo = (v - hi.astype(np.float32)).astype(ml_dtypes.bfloat16)
    return hi, lo


def _head_proj(W, a, cfg):
    """[D, H] with col h = W_h @ a[h]."""
    out = np.zeros((cfg.D, cfg.H), np.float32)
    for h in range(cfg.H):
        out[:, h] = W[:, h * cfg.C:(h + 1) * cfg.C] @ a[h]
    return out


def _hbf_pos(gid, cfg):
    """Map global node id -> row in the split-major allgathered table."""
    k = gid // cfg.SHARD
    r = gid % cfg.SHARD
    pos = np.zeros_like(gid)
    base = 0
    for (s0, sz) in cfg.splits:
        m = (r >= s0) & (r < s0 + sz)
        pos = np.where(m, base + k * sz + (r - s0), pos)
        base += cfg.n_cores * sz
    return pos


def _prep(x, edge_index, W1, a_src1, a_dst1, b1, W2, a_src2, a_dst2, b2, cfg):
    N, SHARD, NBLOCK, H = cfg.N, cfg.SHARD, cfg.NT, cfg.H
    src = np.concatenate([np.asarray(edge_index[0]), np.arange(N)]).astype(np.int64)
    dst = np.concatenate([np.asarray(edge_index[1]), np.arange(N)]).astype(np.int64)
    order = np.argsort(dst, kind="stable")
    src_s, dst_s = src[order], dst[order]

    cores = []
    CPB = 1
    for k in range(cfg.n_cores):
        lo, hi = k * SHARD, (k + 1) * SHARD
        a = np.searchsorted(dst_s, lo)
        b = np.searchsorted(dst_s, hi)
        s_k = src_s[a:b].astype(np.int64)
        d_k = (dst_s[a:b] - lo).astype(np.int64)
        deg = np.bincount(d_k, minlength=SHARD)
        csum = np.concatenate([[0], np.cumsum(deg)])
        for bi in range(NBLOCK):
            n1 = min((bi + 1) * P, SHARD)
            ecnt = int(csum[n1] - csum[bi * P])
            CPB = max(CPB, (ecnt + P - 1) // P)
        cores.append((s_k, d_k, csum))
    NCHUNK = NBLOCK * CPB

    xf = np.asarray(x, np.float32)
    W1f = np.asarray(W1, np.float32)
    W2f = np.asarray(W2, np.float32)
    Wa_s1 = _head_proj(W1f, np.asarray(a_src1, np.float32), cfg)
    Wa_d1 = _head_proj(W1f, np.asarray(a_dst1, np.float32), cfg)
    Wa_s2 = _head_proj(W2f, np.asarray(a_src2, np.float32), cfg)
    Wa_d2 = _head_proj(W2f, np.asarray(a_dst2, np.float32), cfg)

    # layer-1 table: [x | hi_s | lo_s]  (replicated to every core)
    als1 = xf @ Wa_s1                      # [N, H] fp32
    ald1 = xf @ Wa_d1                      # [N, H] fp32
    hi_s, lo_s = _hi_lo(als1)
    T1 = np.concatenate(
        [xf.astype(ml_dtypes.bfloat16), hi_s, lo_s], axis=1)
    T1 = np.ascontiguousarray(T1)          # [N, ROW]

    W2A = np.ascontiguousarray(
        np.concatenate([Wa_s2, Wa_d2], axis=1).astype(ml_dtypes.bfloat16))
    W1K = np.ascontiguousarray(W1f.astype(ml_dtypes.bfloat16))
    W2K = np.ascontiguousarray(W2f.astype(ml_dtypes.bfloat16))
    B1 = np.ascontiguousarray(
        np.broadcast_to(np.asarray(b1, np.float32), (P, cfg.C)))
    B2 = np.ascontiguousarray(
        np.broadcast_to(np.asarray(b2, np.float32), (P, cfg.C)))

    hi_d, lo_d = _hi_lo(ald1)

    in_maps = []
    for k, (s_k, d_k, csum) in enumerate(cores):
        srcg1 = np.zeros((NCHUNK, P), np.int32)
        srcg2 = np.zeros((NCHUNK, P), np.int32)
        MMc = np.zeros((NBLOCK, P, 2 * CPB * P), ml_dtypes.bfloat16)
        s_pos2 = _hbf_pos(s_k, cfg).astype(np.int32)
        s_pos1 = s_k.astype(np.int32)
        for bi in range(NBLOCK):
            n1 = min((bi + 1) * P, SHARD)
            e0, e1_ = int(csum[bi * P]), int(csum[n1])
            ecnt = e1_ - e0
            ed = d_k[e0:e1_] - bi * P
            for srcg, s_pos in ((srcg1, s_pos1), (srcg2, s_pos2)):
                eb = np.zeros(CPB * P, np.int32)
                eb[:ecnt] = s_pos[e0:e1_]
                srcg[bi * CPB:(bi + 1) * CPB] = eb.reshape(CPB, P)
            Mb = np.zeros((CPB * P, P), ml_dtypes.bfloat16)
            Mb[np.arange(ecnt), ed] = 1.0
            Mb3 = Mb.reshape(CPB, P, P)
            MMc[bi, :, :CPB * P] = Mb3.transpose(1, 0, 2).reshape(P, CPB * P)
            MMc[bi, :, CPB * P:] = Mb3.transpose(2, 0, 1).reshape(P, CPB * P)
        # per-block al_dst hi/lo in [slot, block*2H + (hi|lo)] layout
        ALD = np.zeros((P, NBLOCK, 2 * H), ml_dtypes.bfloat16)
        r = np.arange(SHARD)
        ALD[r % P, r // P, :H] = hi_d[k * SHARD:(k + 1) * SHARD]
        ALD[r % P, r // P, H:] = lo_d[k * SHARD:(k + 1) * SHARD]
        in_maps.append({
            "T1": T1,
            "W1K": W1K, "W2K": W2K, "W2A": W2A, "B1": B1, "B2": B2,
            "SRC1": np.ascontiguousarray(srcg1.T),
            "SRC2": np.ascontiguousarray(srcg2.T),
            "ALD1": np.ascontiguousarray(ALD.reshape(P, NBLOCK * 2 * H)),
            "MMC": np.ascontiguousarray(
                MMc.transpose(1, 0, 2).reshape(P, NBLOCK * 2 * CPB * P)),
        })
    return in_maps, CPB


# ---------------------------------------------------------------- device code


def _build(cfg, CPB):
    NBLOCK = cfg.NT
    NCHUNK = NBLOCK * CPB
    D, C, H, HC, ROW = cfg.D, cfg.C, cfg.H, cfg.HC, cfg.ROW
    SHARD, NT, N, KT = cfg.SHARD, cfg.NT, cfg.N, cfg.KT
    MW = 2 * CPB * P
    AW = 2 * H * CPB + H     # attn tile cols: CPB aldt pairs | s

    nc = bacc.Bacc("TRN2", target_bir_lowering=False, debug=False,
                   num_devices=cfg.n_cores)

    T1 = nc.dram_tensor("T1", [N, ROW], BF16, kind="ExternalInput")
    W1K = nc.dram_tensor("W1K", [D, HC], BF16, kind="ExternalInput")
    W2K = nc.dram_tensor("W2K", [D, HC], BF16, kind="ExternalInput")
    W2A = nc.dram_tensor("W2A", [D, 2 * H], BF16, kind="ExternalInput")
    B1 = nc.dram_tensor("B1", [P, C], F32, kind="ExternalInput")
    B2 = nc.dram_tensor("B2", [P, C], F32, kind="ExternalInput")
    SRC1 = nc.dram_tensor("SRC1", [P, NCHUNK], I32, kind="ExternalInput")
    SRC2 = nc.dram_tensor("SRC2", [P, NCHUNK], I32, kind="ExternalInput")
    ALD1 = nc.dram_tensor("ALD1", [P, NBLOCK * 2 * H], BF16, kind="ExternalInput")
    MMC = nc.dram_tensor("MMC", [P, NBLOCK * MW], BF16, kind="ExternalInput")
    OUT = nc.dram_tensor("OUT", [SHARD, C], F32, kind="ExternalOutput")

    hbs2 = nc.dram_tensor("hbs2", [SHARD, ROW], BF16)
    hbf2 = nc.dram_tensor("hbf2", [N, ROW], BF16, addr_space="Shared")

    groups = [list(range(cfg.n_cores))]
    split_rows = cfg.splits
    split_bases = np.cumsum([0] + [cfg.n_cores * sz for (_, sz) in split_rows])
    last_tile_of_split = {ts[-1]: j for j, ts in enumerate(cfg.tile_splits)}

    with tile.TileContext(nc) as tc, ExitStack() as ctx:
        res = ctx.enter_context(tc.tile_pool(name="res", bufs=1))
        b1_sb = res.tile([P, C], F32, name="b1_sb")
        b2_sb = res.tile([P, C], F32, name="b2_sb")
        src1_sb = res.tile([P, NCHUNK], I32, name="src1_sb")
        nc.sync.dma_start(src1_sb[:], SRC1.ap())
        src2_sb = res.tile([P, NCHUNK], I32, name="src2_sb")
        nc.sync.dma_start(src2_sb[:], SRC2.ap())
        ald1 = res.tile([P, NBLOCK * 2 * H], BF16, name="ald1")
        nc.sync.dma_start(ald1[:], ALD1.ap())
        ald2 = res.tile([P, NBLOCK * 2 * H], BF16, name="ald2")
        nc.gpsimd.memset(ald2[:], 0.0)
        w2a_sb = res.tile([P, KT * 2 * H], BF16, name="w2a_sb")
        nc.sync.dma_start(
            w2a_sb[:].rearrange("p (t c) -> p t c", t=KT),
            W2A.ap().rearrange("(t p) c -> p t c", p=P))
        id_sb = res.tile([P, P], BF16, name="id_sb")
        from concourse.masks import make_identity
        make_identity(nc, id_sb[:])

        wp = ctx.enter_context(tc.tile_pool(name="wp", bufs=2))
        xp = ctx.enter_context(tc.tile_pool(name="xp", bufs=2))
        hp = ctx.enter_context(tc.tile_pool(name="hp", bufs=3))
        gp = ctx.enter_context(tc.tile_pool(name="gp", bufs=3))
        mp = ctx.enter_context(tc.tile_pool(name="mp", bufs=3))
        shp = ctx.enter_context(tc.tile_pool(name="shp", bufs=4))
        sp = ctx.enter_context(tc.tile_pool(name="sp", bufs=4))
        op_ = ctx.enter_context(tc.tile_pool(name="op", bufs=2))
        pp = ctx.enter_context(tc.tile_pool(name="pp", bufs=2, space="PSUM"))
        pj = ctx.enter_context(tc.tile_pool(name="pj", bufs=1, space="PSUM"))
        ea = ctx.enter_context(tc.tile_pool(name="ea", bufs=2, space="PSUM"))

        w_sbs = []

        def load_weights(L):
            nc.sync.dma_start((b1_sb if L == 0 else b2_sb)[:],
                              (B1 if L == 0 else B2).ap())
            w_sb = wp.tile([P, KT * HC], BF16, name=f"w_sb{L}", tag="W")
            W = W1K if L == 0 else W2K
            for kt in range(KT):
                nc.sync.dma_start(
                    w_sb[:, kt * HC:(kt + 1) * HC],
                    W.ap()[kt * P:(kt + 1) * P, :])
            w_sbs.append(w_sb)

        aldr_ = [ald1, ald2]
        src_ = [src1_sb, src2_sb]

        class St:
            pass

        def stage_a(L, b):
            """Gather + al_dst expansion + edge logits for block b."""
            st = St()
            st.b = b
            st.Mc = mp.tile([P, MW], BF16, name="Mc", tag="M", bufs=4)
            nc.sync.dma_start(st.Mc[:], MMC.ap()[:, b * MW:(b + 1) * MW])
            st.attn = ea.tile([P, AW], F32, name="attn", tag="attn", bufs=1)
            st.s_ps = ea.tile([P, H], F32, name="s_ps", tag="s", bufs=1)
            aldr = aldr_[L]
            tbl = T1 if L == 0 else hbf2
            st.Gs, st.pfs, st.pbs = [], [], []
            for cc in range(CPB):
                c = b * CPB + cc
                G = gp.tile([P, ROW], BF16, name="G", tag="G", bufs=3 * CPB + 4)
                nc.gpsimd.indirect_dma_start(
                    out=G[:], out_offset=None, in_=tbl.ap(),
                    in_offset=IndirectOffsetOnAxis(
                        ap=src_[L][:, c:c + 1], axis=0))
                ad = st.attn[:, cc * 2 * H:(cc + 1) * 2 * H]
                nc.tensor.matmul(
                    ad, lhsT=st.Mc[:, CPB * P + cc * P: CPB * P + (cc + 1) * P],
                    rhs=aldr[:, b * 2 * H:(b + 1) * 2 * H],
                    start=True, stop=True)
                e1 = sp.tile([P, H], F32, name="e1", tag="e1")
                nc.vector.tensor_tensor(
                    e1[:], G[:, D:D + H], G[:, D + H:D + 2 * H], op=OP.add)
                nc.vector.tensor_tensor(e1[:], e1[:], ad[:, 0:H], op=OP.add)
                nc.vector.tensor_tensor(e1[:], e1[:], ad[:, H:2 * H], op=OP.add)
                nc.vector.scalar_tensor_tensor(
                    e1[:], e1[:], NEG_SLOPE, e1[:], op0=OP.mult, op1=OP.max)
                pf = sp.tile([P, H], F32, name="pf", tag="pf", bufs=3 * CPB + 4)
                nc.scalar.activation(pf[:], e1[:], AF.Exp)
                pb = sp.tile([P, H], BF16, name="pb", tag="pb", bufs=2 * CPB + 2)
                nc.vector.tensor_copy(pb[:], pf[:])
                st.Gs.append(G)
                st.pfs.append(pf)
                st.pbs.append(pb)
            return st

        def stage_s(L, st):
            """Softmax denominator s = M^T p + 1/(3s) for block st.b."""
            s_ap = st.s_ps[:]
            for cc in range(CPB):
                nc.tensor.matmul(
                    s_ap, lhsT=st.Mc[:, cc * P:(cc + 1) * P], rhs=st.pbs[cc][:],
                    start=(cc == 0), stop=(cc == CPB - 1))

        def stage_r(L, st):
            """recip = 1/(3 s) (emitted at the end of the previous B)."""
            st.recip = sp.tile([P, H], F32, name="recip", tag="recip", bufs=2)
            nc.vector.tensor_scalar_mul(st.recip[:], st.s_ps[:], float(H))
            nc.vector.reciprocal(st.recip[:], st.recip[:])

        def stage_b(L, st, nxt, bnd=None):
            """Aggregate (transposed) + project + epilogue for block st.b."""
            b = st.b
            nw = min(P, SHARD - b * P)

            def emit_agg(h):
                # one accumulation group at a time per PSUM bank: kt outer,
                # cc inner (interleaved groups in one bank lose their first
                # chunk -- start=True clears has_written for the whole bank)
                ag = pp.tile([P, KT * P], F32, name="aggT", tag="aggT", bufs=2)
                Shs = []
                for cc in range(CPB):
                    Sh = shp.tile([P, P], BF16, name="Sh", tag="Sh", bufs=6)
                    nc.scalar.activation(
                        Sh[:], st.Mc[:, cc * P:(cc + 1) * P], AF.Copy,
                        scale=st.pfs[cc][:, h:h + 1])
                    Shs.append(Sh)
                for kt in range(KT):
                    for cc in range(CPB):
                        nc.tensor.matmul(
                            ag[:, kt * P:(kt + 1) * P],
                            lhsT=st.Gs[cc][:, kt * P:(kt + 1) * P], rhs=Shs[cc][:],
                            start=(cc == 0), stop=(cc == CPB - 1))
                return ag

            def emit_copy(ag):
                asb = op_.tile([P, KT * P], BF16, name="aggT_sb",
                               tag="aggT_sb", bufs=3)
                nc.scalar.copy(asb[:], ag[:])
                return asb

            def emit_proj(h, asb):
                pA = pj.tile([P, 512], F32, name="pA", tag="pA", bufs=1)
                pB = pj.tile([P, 256], F32, name="pB", tag="pB", bufs=1)
                for (pr, c0, cw) in ((pA, 0, 512), (pB, 512, 256)):
                    for kt in range(KT):
                        nc.tensor.matmul(
                            pr[:nw, :cw],
                            lhsT=asb[:, kt * P:kt * P + nw],
                            rhs=w_sbs[L][:, kt * HC + h * C + c0:
                                         kt * HC + h * C + c0 + cw],
                            start=(kt == 0), stop=(kt == KT - 1))
                return pA, pB

            bias_sb = b1_sb if L == 0 else b2_sb
            o = op_.tile([P, C], F32, name="o", tag="o", bufs=3)

            def emit_stt(h, pA, pB):
                for (pr, c0, cw) in ((pA, 0, 512), (pB, 512, 256)):
                    nc.vector.scalar_tensor_tensor(
                        o[:, c0:c0 + cw], pr[:, :cw], st.recip[:, h:h + 1],
                        bias_sb[:, c0:c0 + cw] if h == 0 else o[:, c0:c0 + cw],
                        op0=OP.mult, op1=OP.add)

            ag0 = emit_agg(0)
            ag1 = emit_agg(1)
            a0 = emit_copy(ag0)
            p0 = emit_proj(0, a0)
            emit_stt(0, *p0)
            ag2 = emit_agg(2)
            if L == 0 and nxt is not None:
                stage_s(L, nxt)
            a1 = emit_copy(ag1)
            p1 = emit_proj(1, a1)
            emit_stt(1, *p1)
            a2 = emit_copy(ag2)
            if L == 1 and nxt is not None:
                stage_s(L, nxt)
            if bnd is not None:
                emit_boundary(bnd)
            p2 = emit_proj(2, a2)
            emit_stt(2, *p2)
            if nxt is not None:
                stage_r(L, nxt)
            st.o = o

            if L == 0 and b >= NBLOCK - 4:
                emit_boundary(st)
            if L == 1:
                nc.sync.dma_start(OUT.ap()[b * P:b * P + nw, :], o[:nw, :])

        def emit_boundary(st):
            """Layer-1 -> layer-2 transition for block st.b (deferred so the
            PE has work queued between proj2/stt2 and the transposes)."""
            b = st.b
            nw = min(P, SHARD - b * P)
            o = st.o
            if True:
                hb = hp.tile([P, ROW], BF16, name="hb", tag="hb", bufs=3)
                nc.scalar.activation(hb[:, 0:D], o[:], AF.Relu)
                lhsT2 = xp.tile([P, KT * P], BF16, name="lhsT2",
                                tag="lhsT2", bufs=2)
                for kt in range(KT):
                    tp = pp.tile([P, 2 * KT * P], BF16, name="tp", tag="aggT",
                                 bufs=2)
                    nc.tensor.transpose(
                        tp[:, 0:P], hb[:, kt * P:(kt + 1) * P], id_sb[:])
                    nc.scalar.copy(
                        lhsT2[:, kt * P:(kt + 1) * P], tp[:, 0:P])
                lg = ea.tile([P, AW], F32, name="lg", tag="attn", bufs=1)
                for kt in range(KT):
                    nc.tensor.matmul(
                        lg[:, 0:2 * H], lhsT=lhsT2[:, kt * P:(kt + 1) * P],
                        rhs=w2a_sb[:, kt * 2 * H:(kt + 1) * 2 * H],
                        start=(kt == 0), stop=(kt == KT - 1))
                nc.vector.tensor_copy(hb[:nw, D:D + H], lg[:nw, 0:H])
                nc.vector.tensor_tensor(
                    hb[:nw, D + H:D + 2 * H], lg[:nw, 0:H],
                    hb[:nw, D:D + H], op=OP.subtract)
                ao = b * 2 * H
                nc.vector.tensor_copy(ald2[:nw, ao:ao + H], lg[:nw, H:2 * H])
                nc.vector.tensor_tensor(
                    ald2[:nw, ao + H:ao + 2 * H], lg[:nw, H:2 * H],
                    ald2[:nw, ao:ao + H], op=OP.subtract)
                nc.sync.dma_start(hbs2.ap()[b * P:b * P + nw, :], hb[:nw, :])
                if b in last_tile_of_split:
                    j = last_tile_of_split[b]
                    s0, sz = split_rows[j]
                    nc.gpsimd.collective_compute(
                        "AllGather", OP.bypass, replica_groups=groups,
                        ins=[hbs2.ap()[s0:s0 + sz, :].opt()],
                        outs=[hbf2.ap()[int(split_bases[j]):
                                        int(split_bases[j + 1]), :].opt()])

        for L in range(2):
            prev, bnd = None, None
            for b in range(NBLOCK):
                cur = stage_a(L, b)
                if L == 0 and b == 0:
                    load_weights(0)
                if L == 0 and b == 2:
                    load_weights(1)
                if prev is None:
                    stage_s(L, cur)
                    stage_r(L, cur)
                else:
                    stage_b(L, prev, cur, bnd)
                    bnd = prev if (L == 0 and prev.b < NBLOCK - 4) else None
                prev = cur
            stage_b(L, prev, None, bnd)

    nc.compile()
    return nc


# ---------------------------------------------------------------- entry point

_NC_CACHE = {}


def _get_nc(cfg, CPB):
    key = (cfg, CPB)
    if key not in _NC_CACHE:
        _NC_CACHE[key] = _build(cfg, CPB)
    return _NC_CACHE[key]


LAST_RUN = {}


def kernel(x, edge_index, W1, a_src1, a_dst1, b1, W2, a_src2, a_dst2, b2,
           cfg=CFG):
    from concourse.bass_utils import run_bass_kernel_spmd

    in_maps, CPB = _prep(x, edge_index, W1, a_src1, a_dst1, b1,
                         W2, a_src2, a_dst2, b2, cfg)
    nc = _get_nc(cfg, CPB)
    trace = os.environ.get("GAT_TRACE", "0") == "1"
    tmpdir = os.environ.get("GAT_TMPDIR") or None
    res = run_bass_kernel_spmd(nc, in_maps, list(range(cfg.n_cores)),
                               trace=trace, tmpdir=tmpdir)
    LAST_RUN["exec_time_ns"] = res.exec_time_ns
    LAST_RUN["profile_json"] = res.profile_json
    out = np.concatenate(
        [res.results[k]["OUT"] for k in range(cfg.n_cores)], 0)
    return np.ascontiguousarray(out.astype(np.float32))


# revision 16
# speedup vs baseline: 1.0181x; 1.0102x over previous
"""Trainium2 Bass kernel for a 2-layer GAT (PyG GATConv, concat=False, 3 heads).

Strategy (8 NeuronCores, SPMD), v2 "aggregate-then-project":
  * The GAT projection is linear, so the weighted message sum commutes with
    it:  sum_e p_e (z_src @ W_h) = (sum_e p_e z_src) @ W_h.  Each core
    therefore aggregates RAW 768-col features per head and applies the
    projection once per destination tile afterwards.  Tensor FLOPs are
    unchanged but the gathered/AllGathered table shrinks 3x (2310 -> 774
    cols): row = [z(768) | al_src hi(3) | al_src lo(3)] bf16.
  * Layer 1's table (x + its attention logits, fp32 on host, hi/lo bf16
    pair) is fully precomputed on the host and staged replicated on every
    core -> layer 1 runs with NO collective at all.  Layer 2's table is
    built on device and AllGathered in NSPLIT row-chunks that overlap the
    layer-1 edge phase.
  * Nodes range-sharded across 8 cores; edges partitioned by destination
    into 128-node dst blocks (CPB padded 128-edge chunks); self-loops
    appended on host.  Per block: indirect-DMA gather of src rows,
    e = leaky_relu(al_s[src] + al_d[dst]) (al_d expanded per edge via a
    mask-transpose matmul), p = exp(e) unnormalized, s = M^T p by mask
    matmul; per head the aggregation is accumulated TRANSPOSED
    (aggT[ch,dst] += G_z_slice^T @ (M*p_h)) so it feeds the projection
    matmul as lhsT with no transposes; out = sum_h (aggT_h^T @ W_h) *
    1/(3 s_h) + bias (normalization and head-mean commute with the sums).
  * PSUM (8 banks): aggT [P,768]f32 x2 bufs (4 banks; the tag is also
    reused for the layer-boundary transposes), projection split into
    512+256 col tiles (2 banks), one "attn" tile per block holding all
    per-chunk al_dst expansions (+ layer-2 logit accumulator via tag
    share, 1 bank), softmax denominator s in its own bank.  Only one
    matmul accumulation group is ever open per bank (start=True clears
    the has_written bits of the WHOLE bank).
  * Software pipeline: stage A (gather/attention) of block b+1 is
    emitted before stage B (aggregate/project) of block b; the s-matmuls
    of b+1 are emitted inside B(b) (after agg2 for layer 1, before
    proj2 for layer 2); the layer-1->2 boundary work of block b (relu,
    transposes, layer-2 logits, table-row DMA, AllGather chunks) is
    deferred into B(b+1) so the PE always has queued work while the
    epilogue drains; head accumulation interleaves with projections.
    Gathers/masks/logit chains prefetch 3-4 blocks ahead.

Self-contained: only numpy/ml_dtypes/concourse (environment packages).
"""

import os
from contextlib import ExitStack
from dataclasses import dataclass

import ml_dtypes
import numpy as np

import concourse.bass as bass
import concourse.mybir as mybir
import concourse.tile as tile
from concourse import bacc
from concourse.bass import IndirectOffsetOnAxis

F32 = mybir.dt.float32
BF16 = mybir.dt.bfloat16
I32 = mybir.dt.int32
AF = mybir.ActivationFunctionType
OP = mybir.AluOpType

P = 128
NEG_SLOPE = 0.2


@dataclass(frozen=True)
class Cfg:
    N: int = 50000           # nodes
    D: int = 768             # feature dim (= per-head channels C)
    H: int = 3               # heads
    C: int = 768             # per-head channels
    n_cores: int = 8
    NSPLIT: int = 7          # layer-2 allgather row-chunks (in node tiles)

    @property
    def HC(self):
        return self.H * self.C

    @property
    def ROW(self):
        return self.D + 2 * self.H  # z | hi_s | lo_s

    @property
    def SHARD(self):
        return self.N // self.n_cores

    @property
    def NT(self):
        return (self.SHARD + P - 1) // P

    @property
    def KT(self):
        return self.D // P

    @property
    def tile_splits(self):
        sizes = [9, 9, 8, 8, 7, 4, 2, 1, 1]
        assert sum(sizes) == self.NT
        out, i = [], 0
        for s in sizes:
            out.append(list(range(i, i + s)))
            i += s
        return out

    @property
    def splits(self):
        out = []
        for t in self.tile_splits:
            r0 = int(t[0]) * P
            r1 = min(int(t[-1] + 1) * P, self.SHARD)
            out.append((r0, r1 - r0))
        return out


CFG = Cfg()


# ---------------------------------------------------------------- host prep


def _hi_lo(v):
    hi = v.astype(ml_dtypes.bfloat16)
    lo = (v - hi.astype(np.float32)).astype(ml_dtypes.bfloat16)
    return hi, lo


def _head_proj(W, a, cfg):
    """[D, H] with col h = W_h @ a[h]."""
    out = np.zeros((cfg.D, cfg.H), np.float32)
    for h in range(cfg.H):
        out[:, h] = W[:, h * cfg.C:(h + 1) * cfg.C] @ a[h]
    return out


def _hbf_pos(gid, cfg):
    """Map global node id -> row in the split-major allgathered table."""
    k = gid // cfg.SHARD
    r = gid % cfg.SHARD
    pos = np.zeros_like(gid)
    base = 0
    for (s0, sz) in cfg.splits:
        m = (r >= s0) & (r < s0 + sz)
        pos = np.where(m, base + k * sz + (r - s0), pos)
        base += cfg.n_cores * sz
    return pos


def _prep(x, edge_index, W1, a_src1, a_dst1, b1, W2, a_src2, a_dst2, b2, cfg):
    N, SHARD, NBLOCK, H = cfg.N, cfg.SHARD, cfg.NT, cfg.H
    src = np.concatenate([np.asarray(edge_index[0]), np.arange(N)]).astype(np.int64)
    dst = np.concatenate([np.asarray(edge_index[1]), np.arange(N)]).astype(np.int64)
    order = np.argsort(dst, kind="stable")
    src_s, dst_s = src[order], dst[order]

    cores = []
    CPB = 1
    for k in range(cfg.n_cores):
        lo, hi = k * SHARD, (k + 1) * SHARD
        a = np.searchsorted(dst_s, lo)
        b = np.searchsorted(dst_s, hi)
        s_k = src_s[a:b].astype(np.int64)
        d_k = (dst_s[a:b] - lo).astype(np.int64)
        deg = np.bincount(d_k, minlength=SHARD)
        csum = np.concatenate([[0], np.cumsum(deg)])
        for bi in range(NBLOCK):
            n1 = min((bi + 1) * P, SHARD)
            ecnt = int(csum[n1] - csum[bi * P])
            CPB = max(CPB, (ecnt + P - 1) // P)
        cores.append((s_k, d_k, csum))
    NCHUNK = NBLOCK * CPB

    xf = np.asarray(x, np.float32)
    W1f = np.asarray(W1, np.float32)
    W2f = np.asarray(W2, np.float32)
    Wa_s1 = _head_proj(W1f, np.asarray(a_src1, np.float32), cfg)
    Wa_d1 = _head_proj(W1f, np.asarray(a_dst1, np.float32), cfg)
    Wa_s2 = _head_proj(W2f, np.asarray(a_src2, np.float32), cfg)
    Wa_d2 = _head_proj(W2f, np.asarray(a_dst2, np.float32), cfg)

    # layer-1 table: [x | hi_s | lo_s]  (replicated to every core)
    als1 = xf @ Wa_s1                      # [N, H] fp32
    ald1 = xf @ Wa_d1                      # [N, H] fp32
    hi_s, lo_s = _hi_lo(als1)
    T1 = np.concatenate(
        [xf.astype(ml_dtypes.bfloat16), hi_s, lo_s], axis=1)
    T1 = np.ascontiguousarray(T1)          # [N, ROW]

    W2A = np.ascontiguousarray(
        np.concatenate([Wa_s2, Wa_d2], axis=1).astype(ml_dtypes.bfloat16))
    W1K = np.ascontiguousarray(W1f.astype(ml_dtypes.bfloat16))
    W2K = np.ascontiguousarray(W2f.astype(ml_dtypes.bfloat16))
    B1 = np.ascontiguousarray(
        np.broadcast_to(np.asarray(b1, np.float32), (P, cfg.C)))
    B2 = np.ascontiguousarray(
        np.broadcast_to(np.asarray(b2, np.float32), (P, cfg.C)))

    hi_d, lo_d = _hi_lo(ald1)

    in_maps = []
    for k, (s_k, d_k, csum) in enumerate(cores):
        srcg1 = np.zeros((NCHUNK, P), np.int32)
        srcg2 = np.zeros((NCHUNK, P), np.int32)
        MMc = np.zeros((NBLOCK, P, 2 * CPB * P), ml_dtypes.bfloat16)
        s_pos2 = _hbf_pos(s_k, cfg).astype(np.int32)
        s_pos1 = s_k.astype(np.int32)
        for bi in range(NBLOCK):
            n1 = min((bi + 1) * P, SHARD)
            e0, e1_ = int(csum[bi * P]), int(csum[n1])
            ecnt = e1_ - e0
            ed = d_k[e0:e1_] - bi * P
            for srcg, s_pos in ((srcg1, s_pos1), (srcg2, s_pos2)):
                eb = np.zeros(CPB * P, np.int32)
                eb[:ecnt] = s_pos[e0:e1_]
                srcg[bi * CPB:(bi + 1) * CPB] = eb.reshape(CPB, P)
            Mb = np.zeros((CPB * P, P), ml_dtypes.bfloat16)
            Mb[np.arange(ecnt), ed] = 1.0
            Mb3 = Mb.reshape(CPB, P, P)
            MMc[bi, :, :CPB * P] = Mb3.transpose(1, 0, 2).reshape(P, CPB * P)
            MMc[bi, :, CPB * P:] = Mb3.transpose(2, 0, 1).reshape(P, CPB * P)
        # per-block al_dst hi/lo in [slot, block*2H + (hi|lo)] layout
        ALD = np.zeros((P, NBLOCK, 2 * H), ml_dtypes.bfloat16)
        r = np.arange(SHARD)
        ALD[r % P, r // P, :H] = hi_d[k * SHARD:(k + 1) * SHARD]
        ALD[r % P, r // P, H:] = lo_d[k * SHARD:(k + 1) * SHARD]
        in_maps.append({
            "T1": T1,
            "W1K": W1K, "W2K": W2K, "W2A": W2A, "B1": B1, "B2": B2,
            "SRC1": np.ascontiguousarray(srcg1.T),
            "SRC2": np.ascontiguousarray(srcg2.T),
            "ALD1": np.ascontiguousarray(ALD.reshape(P, NBLOCK * 2 * H)),
            "MMC": np.ascontiguousarray(
                MMc.transpose(1, 0, 2).reshape(P, NBLOCK * 2 * CPB * P)),
        })
    return in_maps, CPB


# ---------------------------------------------------------------- device code


def _build(cfg, CPB):
    NBLOCK = cfg.NT
    NCHUNK = NBLOCK * CPB
    D, C, H, HC, ROW = cfg.D, cfg.C, cfg.H, cfg.HC, cfg.ROW
    SHARD, NT, N, KT = cfg.SHARD, cfg.NT, cfg.N, cfg.KT
    MW = 2 * CPB * P
    AW = 2 * H * CPB + H     # attn tile cols: CPB aldt pairs | s

    nc = bacc.Bacc("TRN2", target_bir_lowering=False, debug=False,
                   num_devices=cfg.n_cores)

    T1 = nc.dram_tensor("T1", [N, ROW], BF16, kind="ExternalInput")
    W1K = nc.dram_tensor("W1K", [D, HC], BF16, kind="ExternalInput")
    W2K = nc.dram_tensor("W2K", [D, HC], BF16, kind="ExternalInput")
    W2A = nc.dram_tensor("W2A", [D, 2 * H], BF16, kind="ExternalInput")
    B1 = nc.dram_tensor("B1", [P, C], F32, kind="ExternalInput")
    B2 = nc.dram_tensor("B2", [P, C], F32, kind="ExternalInput")
    SRC1 = nc.dram_tensor("SRC1", [P, NCHUNK], I32, kind="ExternalInput")
    SRC2 = nc.dram_tensor("SRC2", [P, NCHUNK], I32, kind="ExternalInput")
    ALD1 = nc.dram_tensor("ALD1", [P, NBLOCK * 2 * H], BF16, kind="ExternalInput")
    MMC = nc.dram_tensor("MMC", [P, NBLOCK * MW], BF16, kind="ExternalInput")
    OUT = nc.dram_tensor("OUT", [SHARD, C], F32, kind="ExternalOutput")

    hbs2 = nc.dram_tensor("hbs2", [SHARD, ROW], BF16)
    hbf2 = nc.dram_tensor("hbf2", [N, ROW], BF16, addr_space="Shared")

    groups = [list(range(cfg.n_cores))]
    split_rows = cfg.splits
    split_bases = np.cumsum([0] + [cfg.n_cores * sz for (_, sz) in split_rows])
    last_tile_of_split = {ts[-1]: j for j, ts in enumerate(cfg.tile_splits)}

    with tile.TileContext(nc) as tc, ExitStack() as ctx:
        res = ctx.enter_context(tc.tile_pool(name="res", bufs=1))
        b1_sb = res.tile([P, C], F32, name="b1_sb")
        b2_sb = res.tile([P, C], F32, name="b2_sb")
        src1_sb = res.tile([P, NCHUNK], I32, name="src1_sb")
        nc.sync.dma_start(src1_sb[:], SRC1.ap())
        src2_sb = res.tile([P, NCHUNK], I32, name="src2_sb")
        nc.sync.dma_start(src2_sb[:], SRC2.ap())
        ald1 = res.tile([P, NBLOCK * 2 * H], BF16, name="ald1")
        nc.sync.dma_start(ald1[:], ALD1.ap())
        ald2 = res.tile([P, NBLOCK * 2 * H], BF16, name="ald2")
        nc.gpsimd.memset(ald2[:], 0.0)
        w2a_sb = res.tile([P, KT * 2 * H], BF16, name="w2a_sb")
        nc.sync.dma_start(
            w2a_sb[:].rearrange("p (t c) -> p t c", t=KT),
            W2A.ap().rearrange("(t p) c -> p t c", p=P))
        id_sb = res.tile([P, P], BF16, name="id_sb")
        from concourse.masks import make_identity
        make_identity(nc, id_sb[:])

        wp = ctx.enter_context(tc.tile_pool(name="wp", bufs=2))
        xp = ctx.enter_context(tc.tile_pool(name="xp", bufs=2))
        hp = ctx.enter_context(tc.tile_pool(name="hp", bufs=3))
        gp = ctx.enter_context(tc.tile_pool(name="gp", bufs=3))
        mp = ctx.enter_context(tc.tile_pool(name="mp", bufs=3))
        shp = ctx.enter_context(tc.tile_pool(name="shp", bufs=4))
        sp = ctx.enter_context(tc.tile_pool(name="sp", bufs=4))
        op_ = ctx.enter_context(tc.tile_pool(name="op", bufs=2))
        pp = ctx.enter_context(tc.tile_pool(name="pp", bufs=2, space="PSUM"))
        pj = ctx.enter_context(tc.tile_pool(name="pj", bufs=1, space="PSUM"))
        ea = ctx.enter_context(tc.tile_pool(name="ea", bufs=2, space="PSUM"))

        w_sbs = []

        def load_weights(L):
            nc.sync.dma_start((b1_sb if L == 0 else b2_sb)[:],
                              (B1 if L == 0 else B2).ap())
            w_sb = wp.tile([P, KT * HC], BF16, name=f"w_sb{L}", tag="W")
            W = W1K if L == 0 else W2K
            for kt in range(KT):
                nc.sync.dma_start(
                    w_sb[:, kt * HC:(kt + 1) * HC],
                    W.ap()[kt * P:(kt + 1) * P, :])
            w_sbs.append(w_sb)

        aldr_ = [ald1, ald2]
        src_ = [src1_sb, src2_sb]

        class St:
            pass

        def stage_a(L, b):
            """Gather + al_dst expansion + edge logits for block b."""
            st = St()
            st.b = b
            st.Mc = mp.tile([P, MW], BF16, name="Mc", tag="M", bufs=4)
            nc.sync.dma_start(st.Mc[:], MMC.ap()[:, b * MW:(b + 1) * MW])
            st.attn = ea.tile([P, AW], F32, name="attn", tag="attn", bufs=1)
            st.s_ps = ea.tile([P, H], F32, name="s_ps", tag="s", bufs=1)
            aldr = aldr_[L]
            tbl = T1 if L == 0 else hbf2
            st.Gs, st.pfs, st.pbs = [], [], []
            for cc in range(CPB):
                c = b * CPB + cc
                G = gp.tile([P, ROW], BF16, name="G", tag="G", bufs=4 * CPB + 4)
                nc.gpsimd.indirect_dma_start(
                    out=G[:], out_offset=None, in_=tbl.ap(),
                    in_offset=IndirectOffsetOnAxis(
                        ap=src_[L][:, c:c + 1], axis=0))
                ad = st.attn[:, cc * 2 * H:(cc + 1) * 2 * H]
                nc.tensor.matmul(
                    ad, lhsT=st.Mc[:, CPB * P + cc * P: CPB * P + (cc + 1) * P],
                    rhs=aldr[:, b * 2 * H:(b + 1) * 2 * H],
                    start=True, stop=True)
                e1 = sp.tile([P, H], F32, name="e1", tag="e1")
                nc.vector.tensor_tensor(
                    e1[:], G[:, D:D + H], G[:, D + H:D + 2 * H], op=OP.add)
                nc.vector.tensor_tensor(e1[:], e1[:], ad[:, 0:H], op=OP.add)
                nc.vector.tensor_tensor(e1[:], e1[:], ad[:, H:2 * H], op=OP.add)
                nc.vector.scalar_tensor_tensor(
                    e1[:], e1[:], NEG_SLOPE, e1[:], op0=OP.mult, op1=OP.max)
                pf = sp.tile([P, H], F32, name="pf", tag="pf", bufs=4 * CPB + 4)
                nc.scalar.activation(pf[:], e1[:], AF.Exp)
                pb = sp.tile([P, H], BF16, name="pb", tag="pb", bufs=3 * CPB + 2)
                nc.vector.tensor_copy(pb[:], pf[:])
                st.Gs.append(G)
                st.pfs.append(pf)
                st.pbs.append(pb)
            return st

        def stage_s(L, st):
            """Softmax denominator s = M^T p + 1/(3s) for block st.b."""
            s_ap = st.s_ps[:]
            for cc in range(CPB):
                nc.tensor.matmul(
                    s_ap, lhsT=st.Mc[:, cc * P:(cc + 1) * P], rhs=st.pbs[cc][:],
                    start=(cc == 0), stop=(cc == CPB - 1))

        def stage_r(L, st):
            """recip = 1/(3 s) (emitted at the end of the previous B)."""
            st.recip = sp.tile([P, H], F32, name="recip", tag="recip", bufs=2)
            nc.vector.tensor_scalar_mul(st.recip[:], st.s_ps[:], float(H))
            nc.vector.reciprocal(st.recip[:], st.recip[:])

        def stage_b(L, st, nxt, bnd=None):
            """Aggregate (transposed) + project + epilogue for block st.b."""
            b = st.b
            nw = min(P, SHARD - b * P)

            def emit_agg(h):
                # one accumulation group at a time per PSUM bank: kt outer,
                # cc inner (interleaved groups in one bank lose their first
                # chunk -- start=True clears has_written for the whole bank)
                ag = pp.tile([P, KT * P], F32, name="aggT", tag="aggT", bufs=2)
                Shs = []
                for cc in range(CPB):
                    Sh = shp.tile([P, P], BF16, name="Sh", tag="Sh", bufs=6)
                    nc.scalar.activation(
                        Sh[:], st.Mc[:, cc * P:(cc + 1) * P], AF.Copy,
                        scale=st.pfs[cc][:, h:h + 1])
                    Shs.append(Sh)
                for kt in range(KT):
                    for cc in range(CPB):
                        nc.tensor.matmul(
                            ag[:, kt * P:(kt + 1) * P],
                            lhsT=st.Gs[cc][:, kt * P:(kt + 1) * P], rhs=Shs[cc][:],
                            start=(cc == 0), stop=(cc == CPB - 1))
                return ag

            def emit_copy(ag):
                asb = op_.tile([P, KT * P], BF16, name="aggT_sb",
                               tag="aggT_sb", bufs=3)
                nc.scalar.copy(asb[:], ag[:])
                return asb

            def emit_proj(h, asb):
                pA = pj.tile([P, 512], F32, name="pA", tag="pA", bufs=1)
                pB = pj.tile([P, 256], F32, name="pB", tag="pB", bufs=1)
                for (pr, c0, cw) in ((pA, 0, 512), (pB, 512, 256)):
                    for kt in range(KT):
                        nc.tensor.matmul(
                            pr[:nw, :cw],
                            lhsT=asb[:, kt * P:kt * P + nw],
                            rhs=w_sbs[L][:, kt * HC + h * C + c0:
                                         kt * HC + h * C + c0 + cw],
                            start=(kt == 0), stop=(kt == KT - 1))
                return pA, pB

            bias_sb = b1_sb if L == 0 else b2_sb
            o = op_.tile([P, C], F32, name="o", tag="o", bufs=3)

            def emit_stt(h, pA, pB):
                for (pr, c0, cw) in ((pA, 0, 512), (pB, 512, 256)):
                    nc.vector.scalar_tensor_tensor(
                        o[:, c0:c0 + cw], pr[:, :cw], st.recip[:, h:h + 1],
                        bias_sb[:, c0:c0 + cw] if h == 0 else o[:, c0:c0 + cw],
                        op0=OP.mult, op1=OP.add)

            ag0 = emit_agg(0)
            ag1 = emit_agg(1)
            a0 = emit_copy(ag0)
            p0 = emit_proj(0, a0)
            emit_stt(0, *p0)
            ag2 = emit_agg(2)
            if L == 0 and nxt is not None:
                stage_s(L, nxt)
            a1 = emit_copy(ag1)
            p1 = emit_proj(1, a1)
            emit_stt(1, *p1)
            a2 = emit_copy(ag2)
            if L == 1 and nxt is not None:
                stage_s(L, nxt)
            if bnd is not None:
                emit_boundary(bnd)
            p2 = emit_proj(2, a2)
            emit_stt(2, *p2)
            if nxt is not None:
                stage_r(L, nxt)
            st.o = o

            if L == 0 and b >= NBLOCK - 4:
                emit_boundary(st)
            if L == 1:
                nc.sync.dma_start(OUT.ap()[b * P:b * P + nw, :], o[:nw, :])

        def emit_boundary(st):
            """Layer-1 -> layer-2 transition for block st.b (deferred so the
            PE has work queued between proj2/stt2 and the transposes)."""
            b = st.b
            nw = min(P, SHARD - b * P)
            o = st.o
            if True:
                hb = hp.tile([P, ROW], BF16, name="hb", tag="hb", bufs=3)
                nc.scalar.activation(hb[:, 0:D], o[:], AF.Relu)
                lhsT2 = xp.tile([P, KT * P], BF16, name="lhsT2",
                                tag="lhsT2", bufs=2)
                for kt in range(KT):
                    tp = pp.tile([P, 2 * KT * P], BF16, name="tp", tag="aggT",
                                 bufs=2)
                    nc.tensor.transpose(
                        tp[:, 0:P], hb[:, kt * P:(kt + 1) * P], id_sb[:])
                    nc.scalar.copy(
                        lhsT2[:, kt * P:(kt + 1) * P], tp[:, 0:P])
                lg = ea.tile([P, AW], F32, name="lg", tag="attn", bufs=1)
                for kt in range(KT):
                    nc.tensor.matmul(
                        lg[:, 0:2 * H], lhsT=lhsT2[:, kt * P:(kt + 1) * P],
                        rhs=w2a_sb[:, kt * 2 * H:(kt + 1) * 2 * H],
                        start=(kt == 0), stop=(kt == KT - 1))
                nc.vector.tensor_copy(hb[:nw, D:D + H], lg[:nw, 0:H])
                nc.vector.tensor_tensor(
                    hb[:nw, D + H:D + 2 * H], lg[:nw, 0:H],
                    hb[:nw, D:D + H], op=OP.subtract)
                ao = b * 2 * H
                nc.vector.tensor_copy(ald2[:nw, ao:ao + H], lg[:nw, H:2 * H])
                nc.vector.tensor_tensor(
                    ald2[:nw, ao + H:ao + 2 * H], lg[:nw, H:2 * H],
                    ald2[:nw, ao:ao + H], op=OP.subtract)
                nc.sync.dma_start(hbs2.ap()[b * P:b * P + nw, :], hb[:nw, :])
                if b in last_tile_of_split:
                    j = last_tile_of_split[b]
                    s0, sz = split_rows[j]
                    nc.gpsimd.collective_compute(
                        "AllGather", OP.bypass, replica_groups=groups,
                        ins=[hbs2.ap()[s0:s0 + sz, :].opt()],
                        outs=[hbf2.ap()[int(split_bases[j]):
                                        int(split_bases[j + 1]), :].opt()])

        for L in range(2):
            prev, bnd = None, None
            for b in range(NBLOCK):
                cur = stage_a(L, b)
                if L == 0 and b == 0:
                    load_weights(0)
                if L == 0 and b == 2:
                    load_weights(1)
                if prev is None:
                    stage_s(L, cur)
                    stage_r(L, cur)
                else:
                    stage_b(L, prev, cur, bnd)
                    bnd = prev if (L == 0 and prev.b < NBLOCK - 4) else None
                prev = cur
            stage_b(L, prev, None, bnd)

    nc.compile()
    return nc


# ---------------------------------------------------------------- entry point

_NC_CACHE = {}


def _get_nc(cfg, CPB):
    key = (cfg, CPB)
    if key not in _NC_CACHE:
        _NC_CACHE[key] = _build(cfg, CPB)
    return _NC_CACHE[key]


LAST_RUN = {}


def kernel(x, edge_index, W1, a_src1, a_dst1, b1, W2, a_src2, a_dst2, b2,
           cfg=CFG):
    from concourse.bass_utils import run_bass_kernel_spmd

    in_maps, CPB = _prep(x, edge_index, W1, a_src1, a_dst1, b1,
                         W2, a_src2, a_dst2, b2, cfg)
    nc = _get_nc(cfg, CPB)
    trace = os.environ.get("GAT_TRACE", "0") == "1"
    tmpdir = os.environ.get("GAT_TMPDIR") or None
    res = run_bass_kernel_spmd(nc, in_maps, list(range(cfg.n_cores)),
                               trace=trace, tmpdir=tmpdir)
    LAST_RUN["exec_time_ns"] = res.exec_time_ns
    LAST_RUN["profile_json"] = res.profile_json
    out = np.concatenate(
        [res.results[k]["OUT"] for k in range(cfg.n_cores)], 0)
    return np.ascontiguousarray(out.astype(np.float32))


# revision 17
# speedup vs baseline: 1.0562x; 1.0374x over previous
"""Trainium2 Bass kernel for a 2-layer GAT (PyG GATConv, concat=False, 3 heads).

Strategy (8 NeuronCores, SPMD), v2 "aggregate-then-project":
  * The GAT projection is linear, so the weighted message sum commutes with
    it:  sum_e p_e (z_src @ W_h) = (sum_e p_e z_src) @ W_h.  Each core
    therefore aggregates RAW 768-col features per head and applies the
    projection once per destination tile afterwards.  Tensor FLOPs are
    unchanged but the gathered/AllGathered table shrinks 3x (2310 -> 774
    cols): row = [z(768) | al_src hi(3) | al_src lo(3)] bf16.
  * Layer 1's table (x + its attention logits, fp32 on host, hi/lo bf16
    pair) is fully precomputed on the host and staged replicated on every
    core -> layer 1 runs with NO collective at all.  Layer 2's table is
    built on device and AllGathered in NSPLIT row-chunks that overlap the
    layer-1 edge phase.
  * Nodes range-sharded across 8 cores; edges partitioned by destination
    into 128-node dst blocks (CPB padded 128-edge chunks); self-loops
    appended on host.  Per block: indirect-DMA gather of src rows,
    e = leaky_relu(al_s[src] + al_d[dst]) (al_d expanded per edge via a
    mask-transpose matmul), p = exp(e) unnormalized, s = M^T p by mask
    matmul; per head the aggregation is accumulated TRANSPOSED
    (aggT[ch,dst] += G_z_slice^T @ (M*p_h)) so it feeds the projection
    matmul as lhsT with no transposes; out = sum_h (aggT_h^T @ W_h) *
    1/(3 s_h) + bias (normalization and head-mean commute with the sums).
  * PSUM (8 banks): aggT [P,768]f32 x2 bufs (4 banks; the tag is also
    reused for the layer-boundary transposes), projection split into
    512+256 col tiles (2 banks), one "attn" tile per block holding all
    per-chunk al_dst expansions (+ layer-2 logit accumulator via tag
    share, 1 bank), softmax denominator s in its own bank.  Only one
    matmul accumulation group is ever open per bank (start=True clears
    the has_written bits of the WHOLE bank).
  * Software pipeline: stage A (gather/attention) of block b+1 is
    emitted before stage B (aggregate/project) of block b; the s-matmuls
    of b+1 are emitted inside B(b) (after agg2 for layer 1, before
    proj2 for layer 2); the layer-1->2 boundary work of block b (relu,
    transposes, layer-2 logits, table-row DMA, AllGather chunks) is
    deferred into B(b+1) so the PE always has queued work while the
    epilogue drains; head accumulation interleaves with projections.
    Gathers/masks/logit chains prefetch 3-4 blocks ahead.

Self-contained: only numpy/ml_dtypes/concourse (environment packages).
"""

import os
from contextlib import ExitStack
from dataclasses import dataclass

import ml_dtypes
import numpy as np

import concourse.bass as bass
import concourse.mybir as mybir
import concourse.tile as tile
from concourse import bacc
from concourse.bass import IndirectOffsetOnAxis

F32 = mybir.dt.float32
BF16 = mybir.dt.bfloat16
I32 = mybir.dt.int32
AF = mybir.ActivationFunctionType
OP = mybir.AluOpType

P = 128
NEG_SLOPE = 0.2


@dataclass(frozen=True)
class Cfg:
    N: int = 50000           # nodes
    D: int = 768             # feature dim (= per-head channels C)
    H: int = 3               # heads
    C: int = 768             # per-head channels
    n_cores: int = 8
    NSPLIT: int = 7          # layer-2 allgather row-chunks (in node tiles)

    @property
    def HC(self):
        return self.H * self.C

    @property
    def ROW(self):
        return self.D + 2 * self.H  # z | hi_s | lo_s

    @property
    def SHARD(self):
        return self.N // self.n_cores

    @property
    def NT(self):
        return (self.SHARD + P - 1) // P

    @property
    def KT(self):
        return self.D // P

    @property
    def tile_splits(self):
        sizes = [9, 9, 8, 8, 7, 4, 3, 1]
        assert sum(sizes) == self.NT
        out, i = [], 0
        for s in sizes:
            out.append(list(range(i, i + s)))
            i += s
        return out

    @property
    def splits(self):
        out = []
        for t in self.tile_splits:
            r0 = int(t[0]) * P
            r1 = min(int(t[-1] + 1) * P, self.SHARD)
            out.append((r0, r1 - r0))
        return out


CFG = Cfg()


# ---------------------------------------------------------------- host prep


def _hi_lo(v):
    hi = v.astype(ml_dtypes.bfloat16)
    lo = (v - hi.astype(np.float32)).astype(ml_dtypes.bfloat16)
    return hi, lo


def _head_proj(W, a, cfg):
    """[D, H] with col h = W_h @ a[h]."""
    out = np.zeros((cfg.D, cfg.H), np.float32)
    for h in range(cfg.H):
        out[:, h] = W[:, h * cfg.C:(h + 1) * cfg.C] @ a[h]
    return out


def _hbf_pos(gid, cfg):
    """Map global node id -> row in the split-major allgathered table."""
    k = gid // cfg.SHARD
    r = gid % cfg.SHARD
    pos = np.zeros_like(gid)
    base = 0
    for (s0, sz) in cfg.splits:
        m = (r >= s0) & (r < s0 + sz)
        pos = np.where(m, base + k * sz + (r - s0), pos)
        base += cfg.n_cores * sz
    return pos


def _prep(x, edge_index, W1, a_src1, a_dst1, b1, W2, a_src2, a_dst2, b2, cfg):
    N, SHARD, NBLOCK, H = cfg.N, cfg.SHARD, cfg.NT, cfg.H
    src = np.concatenate([np.asarray(edge_index[0]), np.arange(N)]).astype(np.int64)
    dst = np.concatenate([np.asarray(edge_index[1]), np.arange(N)]).astype(np.int64)
    order = np.argsort(dst, kind="stable")
    src_s, dst_s = src[order], dst[order]

    cores = []
    CPB = 1
    for k in range(cfg.n_cores):
        lo, hi = k * SHARD, (k + 1) * SHARD
        a = np.searchsorted(dst_s, lo)
        b = np.searchsorted(dst_s, hi)
        s_k = src_s[a:b].astype(np.int64)
        d_k = (dst_s[a:b] - lo).astype(np.int64)
        deg = np.bincount(d_k, minlength=SHARD)
        csum = np.concatenate([[0], np.cumsum(deg)])
        for bi in range(NBLOCK):
            n1 = min((bi + 1) * P, SHARD)
            ecnt = int(csum[n1] - csum[bi * P])
            CPB = max(CPB, (ecnt + P - 1) // P)
        cores.append((s_k, d_k, csum))
    NCHUNK = NBLOCK * CPB

    xf = np.asarray(x, np.float32)
    W1f = np.asarray(W1, np.float32)
    W2f = np.asarray(W2, np.float32)
    Wa_s1 = _head_proj(W1f, np.asarray(a_src1, np.float32), cfg)
    Wa_d1 = _head_proj(W1f, np.asarray(a_dst1, np.float32), cfg)
    Wa_s2 = _head_proj(W2f, np.asarray(a_src2, np.float32), cfg)
    Wa_d2 = _head_proj(W2f, np.asarray(a_dst2, np.float32), cfg)

    # layer-1 table: [x | hi_s | lo_s]  (replicated to every core)
    als1 = xf @ Wa_s1                      # [N, H] fp32
    ald1 = xf @ Wa_d1                      # [N, H] fp32
    hi_s, lo_s = _hi_lo(als1)
    T1 = np.concatenate(
        [xf.astype(ml_dtypes.bfloat16), hi_s, lo_s], axis=1)
    T1 = np.ascontiguousarray(T1)          # [N, ROW]

    W2A = np.ascontiguousarray(
        np.concatenate([Wa_s2, Wa_d2], axis=1).astype(ml_dtypes.bfloat16))
    W1K = np.ascontiguousarray(W1f.astype(ml_dtypes.bfloat16))
    W2K = np.ascontiguousarray(W2f.astype(ml_dtypes.bfloat16))
    B1 = np.ascontiguousarray(
        np.broadcast_to(np.asarray(b1, np.float32), (P, cfg.C)))
    B2 = np.ascontiguousarray(
        np.broadcast_to(np.asarray(b2, np.float32), (P, cfg.C)))

    hi_d, lo_d = _hi_lo(ald1)

    in_maps = []
    for k, (s_k, d_k, csum) in enumerate(cores):
        srcg1 = np.zeros((NCHUNK, P), np.int32)
        srcg2 = np.zeros((NCHUNK, P), np.int32)
        MMc = np.zeros((NBLOCK, P, 2 * CPB * P), ml_dtypes.bfloat16)
        s_pos2 = _hbf_pos(s_k, cfg).astype(np.int32)
        s_pos1 = s_k.astype(np.int32)
        for bi in range(NBLOCK):
            n1 = min((bi + 1) * P, SHARD)
            e0, e1_ = int(csum[bi * P]), int(csum[n1])
            ecnt = e1_ - e0
            ed = d_k[e0:e1_] - bi * P
            for srcg, s_pos in ((srcg1, s_pos1), (srcg2, s_pos2)):
                eb = np.zeros(CPB * P, np.int32)
                eb[:ecnt] = s_pos[e0:e1_]
                srcg[bi * CPB:(bi + 1) * CPB] = eb.reshape(CPB, P)
            Mb = np.zeros((CPB * P, P), ml_dtypes.bfloat16)
            Mb[np.arange(ecnt), ed] = 1.0
            Mb3 = Mb.reshape(CPB, P, P)
            MMc[bi, :, :CPB * P] = Mb3.transpose(1, 0, 2).reshape(P, CPB * P)
            MMc[bi, :, CPB * P:] = Mb3.transpose(2, 0, 1).reshape(P, CPB * P)
        # per-block al_dst hi/lo in [slot, block*2H + (hi|lo)] layout
        ALD = np.zeros((P, NBLOCK, 2 * H), ml_dtypes.bfloat16)
        r = np.arange(SHARD)
        ALD[r % P, r // P, :H] = hi_d[k * SHARD:(k + 1) * SHARD]
        ALD[r % P, r // P, H:] = lo_d[k * SHARD:(k + 1) * SHARD]
        in_maps.append({
            "T1": T1,
            "W1K": W1K, "W2K": W2K, "W2A": W2A, "B1": B1, "B2": B2,
            "SRC1": np.ascontiguousarray(srcg1.T),
            "SRC2": np.ascontiguousarray(srcg2.T),
            "ALD1": np.ascontiguousarray(ALD.reshape(P, NBLOCK * 2 * H)),
            "MMC": np.ascontiguousarray(
                MMc.transpose(1, 0, 2).reshape(P, NBLOCK * 2 * CPB * P)),
        })
    return in_maps, CPB


# ---------------------------------------------------------------- device code


def _build(cfg, CPB):
    NBLOCK = cfg.NT
    NCHUNK = NBLOCK * CPB
    D, C, H, HC, ROW = cfg.D, cfg.C, cfg.H, cfg.HC, cfg.ROW
    SHARD, NT, N, KT = cfg.SHARD, cfg.NT, cfg.N, cfg.KT
    MW = 2 * CPB * P
    AW = 2 * H * CPB + H     # attn tile cols: CPB aldt pairs | s

    nc = bacc.Bacc("TRN2", target_bir_lowering=False, debug=False,
                   num_devices=cfg.n_cores)

    T1 = nc.dram_tensor("T1", [N, ROW], BF16, kind="ExternalInput")
    W1K = nc.dram_tensor("W1K", [D, HC], BF16, kind="ExternalInput")
    W2K = nc.dram_tensor("W2K", [D, HC], BF16, kind="ExternalInput")
    W2A = nc.dram_tensor("W2A", [D, 2 * H], BF16, kind="ExternalInput")
    B1 = nc.dram_tensor("B1", [P, C], F32, kind="ExternalInput")
    B2 = nc.dram_tensor("B2", [P, C], F32, kind="ExternalInput")
    SRC1 = nc.dram_tensor("SRC1", [P, NCHUNK], I32, kind="ExternalInput")
    SRC2 = nc.dram_tensor("SRC2", [P, NCHUNK], I32, kind="ExternalInput")
    ALD1 = nc.dram_tensor("ALD1", [P, NBLOCK * 2 * H], BF16, kind="ExternalInput")
    MMC = nc.dram_tensor("MMC", [P, NBLOCK * MW], BF16, kind="ExternalInput")
    OUT = nc.dram_tensor("OUT", [SHARD, C], F32, kind="ExternalOutput")

    hbs2 = nc.dram_tensor("hbs2", [SHARD, ROW], BF16)
    hbf2 = nc.dram_tensor("hbf2", [N, ROW], BF16, addr_space="Shared")

    groups = [list(range(cfg.n_cores))]
    split_rows = cfg.splits
    split_bases = np.cumsum([0] + [cfg.n_cores * sz for (_, sz) in split_rows])
    last_tile_of_split = {ts[-1]: j for j, ts in enumerate(cfg.tile_splits)}

    with tile.TileContext(nc) as tc, ExitStack() as ctx:
        res = ctx.enter_context(tc.tile_pool(name="res", bufs=1))
        b1_sb = res.tile([P, C], F32, name="b1_sb")
        b2_sb = res.tile([P, C], F32, name="b2_sb")
        src1_sb = res.tile([P, NCHUNK], I32, name="src1_sb")
        nc.sync.dma_start(src1_sb[:], SRC1.ap())
        src2_sb = res.tile([P, NCHUNK], I32, name="src2_sb")
        nc.sync.dma_start(src2_sb[:], SRC2.ap())
        ald1 = res.tile([P, NBLOCK * 2 * H], BF16, name="ald1")
        nc.sync.dma_start(ald1[:], ALD1.ap())
        ald2 = res.tile([P, NBLOCK * 2 * H], BF16, name="ald2")
        nc.gpsimd.memset(ald2[:], 0.0)
        w2a_sb = res.tile([P, KT * 2 * H], BF16, name="w2a_sb")
        nc.sync.dma_start(
            w2a_sb[:].rearrange("p (t c) -> p t c", t=KT),
            W2A.ap().rearrange("(t p) c -> p t c", p=P))
        id_sb = res.tile([P, P], BF16, name="id_sb")
        from concourse.masks import make_identity
        make_identity(nc, id_sb[:])

        wp = ctx.enter_context(tc.tile_pool(name="wp", bufs=2))
        xp = ctx.enter_context(tc.tile_pool(name="xp", bufs=2))
        hp = ctx.enter_context(tc.tile_pool(name="hp", bufs=3))
        gp = ctx.enter_context(tc.tile_pool(name="gp", bufs=3))
        mp = ctx.enter_context(tc.tile_pool(name="mp", bufs=3))
        shp = ctx.enter_context(tc.tile_pool(name="shp", bufs=4))
        sp = ctx.enter_context(tc.tile_pool(name="sp", bufs=4))
        op_ = ctx.enter_context(tc.tile_pool(name="op", bufs=2))
        pp = ctx.enter_context(tc.tile_pool(name="pp", bufs=2, space="PSUM"))
        pj = ctx.enter_context(tc.tile_pool(name="pj", bufs=1, space="PSUM"))
        ea = ctx.enter_context(tc.tile_pool(name="ea", bufs=2, space="PSUM"))

        w_sbs = []

        def load_weights(L):
            nc.sync.dma_start((b1_sb if L == 0 else b2_sb)[:],
                              (B1 if L == 0 else B2).ap())
            w_sb = wp.tile([P, KT * HC], BF16, name=f"w_sb{L}", tag="W")
            W = W1K if L == 0 else W2K
            for kt in range(KT):
                nc.sync.dma_start(
                    w_sb[:, kt * HC:(kt + 1) * HC],
                    W.ap()[kt * P:(kt + 1) * P, :])
            w_sbs.append(w_sb)

        aldr_ = [ald1, ald2]
        src_ = [src1_sb, src2_sb]

        class St:
            pass

        def stage_a(L, b):
            """Gather + al_dst expansion + edge logits for block b."""
            st = St()
            st.b = b
            st.Mc = mp.tile([P, MW], BF16, name="Mc", tag="M", bufs=4)
            nc.sync.dma_start(st.Mc[:], MMC.ap()[:, b * MW:(b + 1) * MW])
            st.attn = ea.tile([P, AW], F32, name="attn", tag="attn", bufs=1)
            st.s_ps = ea.tile([P, H], F32, name="s_ps", tag="s", bufs=1)
            aldr = aldr_[L]
            tbl = T1 if L == 0 else hbf2
            st.Gs, st.pfs, st.pbs = [], [], []
            for cc in range(CPB):
                c = b * CPB + cc
                G = gp.tile([P, ROW], BF16, name="G", tag="G", bufs=3 * CPB + 4)
                nc.gpsimd.indirect_dma_start(
                    out=G[:], out_offset=None, in_=tbl.ap(),
                    in_offset=IndirectOffsetOnAxis(
                        ap=src_[L][:, c:c + 1], axis=0))
                ad = st.attn[:, cc * 2 * H:(cc + 1) * 2 * H]
                nc.tensor.matmul(
                    ad, lhsT=st.Mc[:, CPB * P + cc * P: CPB * P + (cc + 1) * P],
                    rhs=aldr[:, b * 2 * H:(b + 1) * 2 * H],
                    start=True, stop=True)
                e1 = sp.tile([P, H], F32, name="e1", tag="e1")
                nc.vector.tensor_tensor(
                    e1[:], G[:, D:D + H], G[:, D + H:D + 2 * H], op=OP.add)
                nc.vector.tensor_tensor(e1[:], e1[:], ad[:, 0:H], op=OP.add)
                nc.vector.tensor_tensor(e1[:], e1[:], ad[:, H:2 * H], op=OP.add)
                nc.vector.scalar_tensor_tensor(
                    e1[:], e1[:], NEG_SLOPE, e1[:], op0=OP.mult, op1=OP.max)
                pf = sp.tile([P, H], F32, name="pf", tag="pf", bufs=3 * CPB + 4)
                nc.scalar.activation(pf[:], e1[:], AF.Exp)
                pb = sp.tile([P, H], BF16, name="pb", tag="pb", bufs=2 * CPB + 2)
                nc.vector.tensor_copy(pb[:], pf[:])
                st.Gs.append(G)
                st.pfs.append(pf)
                st.pbs.append(pb)
            return st

        def stage_s(L, st):
            """Softmax denominator s = M^T p + 1/(3s) for block st.b."""
            s_ap = st.s_ps[:]
            for cc in range(CPB):
                nc.tensor.matmul(
                    s_ap, lhsT=st.Mc[:, cc * P:(cc + 1) * P], rhs=st.pbs[cc][:],
                    start=(cc == 0), stop=(cc == CPB - 1))

        def stage_r(L, st):
            """recip = 1/(3 s) (emitted at the end of the previous B)."""
            st.recip = sp.tile([P, H], F32, name="recip", tag="recip", bufs=2)
            nc.vector.tensor_scalar_mul(st.recip[:], st.s_ps[:], float(H))
            nc.vector.reciprocal(st.recip[:], st.recip[:])

        def stage_b(L, st, nxt, bnd=None):
            """Aggregate (transposed) + project + epilogue for block st.b."""
            b = st.b
            nw = min(P, SHARD - b * P)

            def emit_agg(h):
                # one accumulation group at a time per PSUM bank: kt outer,
                # cc inner (interleaved groups in one bank lose their first
                # chunk -- start=True clears has_written for the whole bank)
                ag = pp.tile([P, KT * P], F32, name="aggT", tag="aggT", bufs=2)
                Shs = []
                for cc in range(CPB):
                    Sh = shp.tile([P, P], BF16, name="Sh", tag="Sh", bufs=6)
                    nc.scalar.activation(
                        Sh[:], st.Mc[:, cc * P:(cc + 1) * P], AF.Copy,
                        scale=st.pfs[cc][:, h:h + 1])
                    Shs.append(Sh)
                for kt in range(KT):
                    for cc in range(CPB):
                        nc.tensor.matmul(
                            ag[:, kt * P:(kt + 1) * P],
                            lhsT=st.Gs[cc][:, kt * P:(kt + 1) * P], rhs=Shs[cc][:],
                            start=(cc == 0), stop=(cc == CPB - 1))
                return ag

            def emit_copy(ag):
                asb = op_.tile([P, KT * P], BF16, name="aggT_sb",
                               tag="aggT_sb", bufs=3)
                nc.scalar.copy(asb[:], ag[:])
                return asb

            def emit_proj(h, asb):
                pA = pj.tile([P, 512], F32, name="pA", tag="pA", bufs=1)
                pB = pj.tile([P, 256], F32, name="pB", tag="pB", bufs=1)
                for (pr, c0, cw) in ((pA, 0, 512), (pB, 512, 256)):
                    for kt in range(KT):
                        nc.tensor.matmul(
                            pr[:nw, :cw],
                            lhsT=asb[:, kt * P:kt * P + nw],
                            rhs=w_sbs[L][:, kt * HC + h * C + c0:
                                         kt * HC + h * C + c0 + cw],
                            start=(kt == 0), stop=(kt == KT - 1))
                return pA, pB

            bias_sb = b1_sb if L == 0 else b2_sb
            o = op_.tile([P, C], F32, name="o", tag="o", bufs=3)

            def emit_stt(h, pA, pB):
                for (pr, c0, cw) in ((pA, 0, 512), (pB, 512, 256)):
                    nc.vector.scalar_tensor_tensor(
                        o[:, c0:c0 + cw], pr[:, :cw], st.recip[:, h:h + 1],
                        bias_sb[:, c0:c0 + cw] if h == 0 else o[:, c0:c0 + cw],
                        op0=OP.mult, op1=OP.add)

            ag0 = emit_agg(0)
            ag1 = emit_agg(1)
            a0 = emit_copy(ag0)
            p0 = emit_proj(0, a0)
            emit_stt(0, *p0)
            ag2 = emit_agg(2)
            if L == 0 and nxt is not None:
                stage_s(L, nxt)
            a1 = emit_copy(ag1)
            p1 = emit_proj(1, a1)
            emit_stt(1, *p1)
            a2 = emit_copy(ag2)
            if L == 1 and nxt is not None:
                stage_s(L, nxt)
            if bnd is not None:
                emit_boundary(bnd)
            p2 = emit_proj(2, a2)
            emit_stt(2, *p2)
            if nxt is not None:
                stage_r(L, nxt)
            st.o = o

            if L == 0 and b >= NBLOCK - 4:
                emit_boundary(st)
            if L == 1:
                nc.sync.dma_start(OUT.ap()[b * P:b * P + nw, :], o[:nw, :])

        def emit_boundary(st):
            """Layer-1 -> layer-2 transition for block st.b (deferred so the
            PE has work queued between proj2/stt2 and the transposes)."""
            b = st.b
            nw = min(P, SHARD - b * P)
            o = st.o
            if True:
                hb = hp.tile([P, ROW], BF16, name="hb", tag="hb", bufs=3)
                nc.scalar.activation(hb[:, 0:D], o[:], AF.Relu)
                lhsT2 = xp.tile([P, KT * P], BF16, name="lhsT2",
                                tag="lhsT2", bufs=2)
                for kt in range(KT):
                    tp = pp.tile([P, 2 * KT * P], BF16, name="tp", tag="aggT",
                                 bufs=2)
                    nc.tensor.transpose(
                        tp[:, 0:P], hb[:, kt * P:(kt + 1) * P], id_sb[:])
                    nc.scalar.copy(
                        lhsT2[:, kt * P:(kt + 1) * P], tp[:, 0:P])
                lg = ea.tile([P, AW], F32, name="lg", tag="attn", bufs=1)
                for kt in range(KT):
                    nc.tensor.matmul(
                        lg[:, 0:2 * H], lhsT=lhsT2[:, kt * P:(kt + 1) * P],
                        rhs=w2a_sb[:, kt * 2 * H:(kt + 1) * 2 * H],
                        start=(kt == 0), stop=(kt == KT - 1))
                nc.vector.tensor_copy(hb[:nw, D:D + H], lg[:nw, 0:H])
                nc.vector.tensor_tensor(
                    hb[:nw, D + H:D + 2 * H], lg[:nw, 0:H],
                    hb[:nw, D:D + H], op=OP.subtract)
                ao = b * 2 * H
                nc.vector.tensor_copy(ald2[:nw, ao:ao + H], lg[:nw, H:2 * H])
                nc.vector.tensor_tensor(
                    ald2[:nw, ao + H:ao + 2 * H], lg[:nw, H:2 * H],
                    ald2[:nw, ao:ao + H], op=OP.subtract)
                nc.sync.dma_start(hbs2.ap()[b * P:b * P + nw, :], hb[:nw, :])
                if b in last_tile_of_split:
                    j = last_tile_of_split[b]
                    s0, sz = split_rows[j]
                    nc.gpsimd.collective_compute(
                        "AllGather", OP.bypass, replica_groups=groups,
                        ins=[hbs2.ap()[s0:s0 + sz, :].opt()],
                        outs=[hbf2.ap()[int(split_bases[j]):
                                        int(split_bases[j + 1]), :].opt()])

        for L in range(2):
            prev, bnd = None, None
            for b in range(NBLOCK):
                cur = stage_a(L, b)
                if L == 0 and b == 0:
                    load_weights(0)
                if L == 0 and b == 2:
                    load_weights(1)
                if prev is None:
                    stage_s(L, cur)
                    stage_r(L, cur)
                else:
                    stage_b(L, prev, cur, bnd)
                    bnd = prev if (L == 0 and prev.b < NBLOCK - 4) else None
                prev = cur
            stage_b(L, prev, None, bnd)

    nc.compile()
    return nc


# ---------------------------------------------------------------- entry point

_NC_CACHE = {}


def _get_nc(cfg, CPB):
    key = (cfg, CPB)
    if key not in _NC_CACHE:
        _NC_CACHE[key] = _build(cfg, CPB)
    return _NC_CACHE[key]


LAST_RUN = {}


def kernel(x, edge_index, W1, a_src1, a_dst1, b1, W2, a_src2, a_dst2, b2,
           cfg=CFG):
    from concourse.bass_utils import run_bass_kernel_spmd

    in_maps, CPB = _prep(x, edge_index, W1, a_src1, a_dst1, b1,
                         W2, a_src2, a_dst2, b2, cfg)
    nc = _get_nc(cfg, CPB)
    trace = os.environ.get("GAT_TRACE", "0") == "1"
    tmpdir = os.environ.get("GAT_TMPDIR") or None
    res = run_bass_kernel_spmd(nc, in_maps, list(range(cfg.n_cores)),
                               trace=trace, tmpdir=tmpdir)
    LAST_RUN["exec_time_ns"] = res.exec_time_ns
    LAST_RUN["profile_json"] = res.profile_json
    out = np.concatenate(
        [res.results[k]["OUT"] for k in range(cfg.n_cores)], 0)
    return np.ascontiguousarray(out.astype(np.float32))
